# revision 13
# baseline (speedup 1.0000x reference)
"""DA-RNN (dual-stage attention RNN) forward pass on 8 TRN2 NeuronCores.

Data-parallel: batch 2048 sharded 256 per core, weights replicated.

Algebraic structure exploited (validated against the reference in numpy):
  * Both attention blocks add their state-dependent term as a per-sample
    constant across the softmax axis, so softmax cancels it.  The encoder
    input attention (a1, a2) and the decoder temporal attention (beta) are
    input-only precomputes, and the decoder context vector is constant
    across decoder steps.
  * context only enters through dot products (fc_W, fc_final_W): three
    matvec columns [v, fcW1, w_c] against X_encoded give score/q/r per
    (sample, t); softmax-weighted sums of q, r give the decoder LSTM input
    offset (A) and the output contribution.
  * The decoder LSTM input is scalar per sample.  It is fed as a K=3
    augmented matmul with rhs rows [w_y*y_t; A; 1] against lhsT rows
    [Wd0; Wd0; bd + Wd0*fc_b]: the y rows and ones rows are input-only and
    are staged into the 4 row-tiled strips BEFORE the encoder runs; only
    the single A row (shared by all steps) is written after the encoder,
    so the encoder->decoder transition has a ~2us serial path instead of a
    serialized 36-DMA storm.
  * softmax exp is computed via the sigmoid identity e^x = s/(1-s)
    (scores are O(0.3)), so the Scalar engine never swaps activation
    tables (Exp <-> Sigmoid/Tanh swaps cost 1.3us each on the serial
    path).

Precision plan (validated numerically on the fixed inputs):
  * All hidden (recurrent) weights are fp8e4 (halves weight DMA);
    states stay fp16 and stream as the matmul moving operand (mixed
    fp8-weight x fp16-state matmuls run at full bf16 rate and register as
    activity to the HAM PE clock gate, keeping the array at 2.4 GHz).
  * Input matmuls (K=17/16/3) are packed 4x concurrent via row-tiled
    tile_position strips with replicated weights/rhs.
  * Within a PSUM bank, each accumulation group's chain completes before
    the next group's start=True (a start clears has_written bank-wide).

HAM discipline: the PE clock gate needs sustained activity; dummy/dep-
gated warm matmuls bridge every >3.4us PE-idle window (DMA preamble,
attention precompute, encoder->decoder transition).

On-chip layout: feature-major [dim on partitions (128-chunks), batch on
free].  LSTM states stay in that layout so no transposes in the
recurrences.  PSUM accumulates fp32 throughout.
"""

import sys
import os

sys.path.insert(0, "/opt/trn_rl_repo")
os.environ.setdefault("MYCRO_LOCAL_CACHE", "1")

from contextlib import ExitStack

import numpy as np
import ml_dtypes

import concourse.bass as bass
import concourse.mybir as mybir
import concourse.tile as tile
from concourse import bacc
from concourse.bass_utils import run_bass_kernel_spmd
from concourse.masks import make_identity

F32 = mybir.dt.float32
F16 = mybir.dt.float16
F8 = mybir.dt.float8e4
AF = mybir.ActivationFunctionType
ALU = mybir.AluOpType

NCORES = 8
P = 128
BS = 256          # batch per core
NJ = 2            # 128-partition batch chunks
T = 9             # recurrence steps (T-1 in the reference)
H = 512
IN2 = 15
ME = 16           # encoder gate chunks (4H/128)
KE = 4            # encoder hidden chunks (H/128)
MD = 32           # decoder gate chunks (8H/128)
KD = 8            # decoder hidden chunks (2H/128)

N_WARM_PREFIX = 12
T_PRE = T         # x-tilde transpose steps done before the recurrence


def _np(a):
    return np.asarray(a, dtype=np.float32)


def _f16(a):
    return np.ascontiguousarray(np.asarray(a, dtype=np.float32).astype(np.float16))


def _f8(a):
    return np.ascontiguousarray(
        np.asarray(a, dtype=np.float32).astype(ml_dtypes.float8_e4m3))


def _pack_weights(inp):
    """Host-side weight folding (weight-only transforms; no input math)."""
    Wih1, Whh1 = _np(inp["enc_lstm_Wih"]), _np(inp["enc_lstm_Whh"])
    b1 = _np(inp["enc_lstm_bih"]) + _np(inp["enc_lstm_bhh"])
    Wih2, Whh2 = _np(inp["enc_lstm1_Wih"]), _np(inp["enc_lstm1_Whh"])
    b2 = _np(inp["enc_lstm1_bih"]) + _np(inp["enc_lstm1_bhh"])
    Wd_ih, Wd_hh = _np(inp["dec_lstm_Wih"]), _np(inp["dec_lstm_Whh"])
    bd = _np(inp["dec_lstm_bih"]) + _np(inp["dec_lstm_bhh"])
    attn1_W = _np(inp["dec_attn1_W"])
    attn2_w = _np(inp["dec_attn2_W"])[0]
    fc_W = _np(inp["fc_W"])[0]
    fcf_W = _np(inp["fc_final_W"])[0]

    W1x = attn1_W[:, 4 * H:]                        # (512, 1024)
    v = W1x.T @ attn2_w                             # (1024,)
    fcW1 = fc_W[:2 * H]
    w_c = fcf_W[2 * H:]
    w_d = fcf_W[:2 * H]

    # input+bias weights, replicated at 4 row strips for row-tiled quads
    wia1 = np.concatenate([Wih1.T, b1[None, :]], axis=0)            # (17, 2048)
    wia2 = np.concatenate([Wih2.T, b2[None, :]], axis=0)            # (16, 2048)
    Wd0 = Wd_ih[:, 0]
    wid3 = np.stack([Wd0, Wd0,
                     bd + Wd0 * float(_np(inp["fc_b"])[0])],
                    axis=0)                                         # (3, 4096)

    def _part(w, k):  # (k*P, m) -> (P, k, m) partition-contiguous
        m = w.shape[1]
        return np.ascontiguousarray(
            w.reshape(k, P, m).transpose(1, 0, 2))

    weights = {
        "wia1": _f16(np.broadcast_to(wia1, (4, 17, 4 * H))),
        "wia2": _f16(np.broadcast_to(wia2, (4, 16, 4 * H))),
        "wid3": _f16(np.broadcast_to(wid3, (4, 3, 8 * H))),
        "whh1": _f8(_part(Whh1.T, KE)),                             # (P, KE, 2048)
        "whh2": _f8(_part(Whh2.T, KE)),                             # (P, KE, 2048)
        "whhd": _f8(_part(Wd_hh.T, KD)),                            # (P, KD, 4096)
        "v3": _f16(_part(np.stack([v, fcW1, w_c], axis=1), KD)),    # (P, KD, 3)
        "wd": _f16(_part(w_d[:, None], KD)),                        # (P, KD, 1)
    }
    scalars = {
        "Wf": [float(x) for x in _np(inp["enc_attn_W"])[0, 2 * H:]],    # 9 floats
        "w_y": float(fc_W[2 * H]),
        "fcf_b": float(_np(inp["fc_final_b"])[0]),
    }
    return weights, scalars


def _build(scal, upto="full"):
    nc = bacc.Bacc()

    xd = nc.declare_dram_parameter("x", [BS, T, IN2], F32, isOutput=False)
    yd = nc.declare_dram_parameter("y", [BS, T], F32, isOutput=False)
    wia1d = nc.declare_dram_parameter("wia1", [4, 17, 4 * H], F16, isOutput=False)
    wia2d = nc.declare_dram_parameter("wia2", [4, 16, 4 * H], F16, isOutput=False)
    wid3d = nc.declare_dram_parameter("wid3", [4, 3, 8 * H], F16, isOutput=False)
    whh1d = nc.declare_dram_parameter("whh1", [P, KE, 4 * H], F8, isOutput=False)
    whh2d = nc.declare_dram_parameter("whh2", [P, KE, 4 * H], F8, isOutput=False)
    whhdd = nc.declare_dram_parameter("whhd", [P, KD, 8 * H], F8, isOutput=False)
    v3d = nc.declare_dram_parameter("v3", [P, KD, 3], F16, isOutput=False)
    wdd = nc.declare_dram_parameter("wd", [P, KD, 1], F16, isOutput=False)
    outd = nc.declare_dram_parameter("out", [BS, 1], F32, isOutput=True)
    dbgd = (nc.declare_dram_parameter("dbg", [BS, T, 3], F32, isOutput=True)
            if upto == "enc" else None)

    Wf = scal["Wf"]

    with ExitStack() as ctx:
        tc = ctx.enter_context(tile.TileContext(nc))
        # persistent pools
        pw = ctx.enter_context(tc.tile_pool(name="pw", bufs=1))
        psm = ctx.enter_context(tc.tile_pool(name="psm", bufs=4))     # small f32 scratch
        pu = ctx.enter_context(tc.tile_pool(name="pu", bufs=4))       # cell temp
        pya = ctx.enter_context(tc.tile_pool(name="pya", bufs=1))
        psum_g = ctx.enter_context(tc.tile_pool(name="psum_g", bufs=5, space="PSUM"))
        psum_t = ctx.enter_context(tc.tile_pool(name="psum_t", bufs=1, space="PSUM"))
        psum_q = ctx.enter_context(tc.tile_pool(name="psum_q", bufs=2, space="PSUM"))

        # ---------------- input DMAs (critical path first) ----------------
        yb = pw.tile([P, NJ, T], F32)
        nc.sync.dma_start(out=yb, in_=yd.rearrange("(j p) t -> p j t", p=P))
        xb0 = pw.tile([P, NJ, T, IN2], F32)
        xd_r = xd.rearrange("(j p) t f -> p j t f", p=P)
        for j in range(NJ):
            nc.sync.dma_start(out=xb0[:, j, :, :], in_=xd_r[:, j, :, :])
        v3t = pw.tile([P, KD, 3], F16)
        nc.sync.dma_start(out=v3t, in_=v3d[:, :, :])
        wdt = pw.tile([P, KD, 1], F16)
        nc.sync.dma_start(out=wdt, in_=wdd[:, :, :])
        wiaS1 = pw.tile([P, 4 * H], F16)
        wiaS2 = pw.tile([P, 4 * H], F16)
        for g in range(4):
            nc.sync.dma_start(out=wiaS1[32 * g:32 * g + 17, :], in_=wia1d[g, :, :])
            nc.sync.dma_start(out=wiaS2[32 * g:32 * g + 16, :], in_=wia2d[g, :, :])

        # hidden weights on the gpsimd queue: whh1/whh2 first (needed at
        # encoder t=1), then the decoder weights (needed ~200us later)
        whh1 = pw.tile([P, KE, 4 * H], F8)
        nc.gpsimd.dma_start(out=whh1, in_=whh1d[:, :, :])
        whh2 = pw.tile([P, KE, 4 * H], F8)
        nc.gpsimd.dma_start(out=whh2, in_=whh2d[:, :, :])
        wid3R = pw.tile([P, 8 * H], F16)
        for g in range(4):
            nc.gpsimd.dma_start(out=wid3R[32 * g:32 * g + 3, :], in_=wid3d[g, :, :])
        whhd = pw.tile([P, KD, 8 * H], F8)
        nc.gpsimd.dma_start(out=whhd, in_=whhdd[:, :, :])

        ident = pw.tile([P, P], F16)
        make_identity(nc, ident)
        wbig = pw.tile([P, 480], F16)
        nc.vector.memset(wbig, 1.0)

        # PE warm-up helpers.  The HAM clock gate needs sustained activity;
        # _warm is un-gated (runs immediately at its queue position),
        # _warm_on is gated on a data tile so it fires mid-pipeline.
        def _warm(n):
            for _ in range(n):
                wps = psum_q.tile([P, P], F32, name="wps", tag="psq")
                nc.tensor.matmul(wps, ident, ident, start=True, stop=True)

        def _wfill(n):
            # dense filler: 480-col streams keep the HAM activity monitor
            # fed (un-throttle needs ~27us of SUSTAINED busy on this part,
            # so every idle window must be prevented, not repaired)
            for _ in range(n):
                wps = psum_q.tile([P, 480], F32, name="wf", tag="psq")
                nc.tensor.matmul(wps, ident, wbig, start=True, stop=True)

        wseq = [0]

        def _warm_on(lhs, rhs, n=1):
            m = lhs.free_size()
            nfree = rhs.free_size()
            for _ in range(n):
                wseq[0] += 1
                wps = psum_q.tile([min(m, P), min(nfree, 480)], F32,
                                  name=f"wo{wseq[0]}", tag="psq")
                nc.tensor.matmul(wps, lhs, rhs, start=True, stop=True)

        _wfill(6)

        # persistent state / small tiles
        sq2 = pw.tile([P, NJ, T, 2], F32)     # q, r matvec results
        es = pw.tile([P, NJ, T], F32)         # sigmoid(score) per step
        A_t = pw.tile([P, NJ], F32)
        A16 = pw.tile([P, NJ], F16)
        ctxw = pw.tile([P, NJ], F32)
        ytldT3 = pw.tile([3 * T, BS], F16)    # [w_y*y_t; 0; 1] transposed
        arowT = pw.tile([1, BS], F16)         # A row, post-encoder
        cd = pw.tile([P, KD, BS], F32)
        osb = pw.tile([P, NJ, 1], F32)
        ytA = [pya.tile([P, BS], F16, name=f"ytA{t}", tag=f"ytA{t}")
               for t in range(T)]

        # ---------------- decoder input rows (pre-encoder part) -----------
        # ytld3 free layout f = 3t + r with rows [w_y*y_t; A=0; 1]; the
        # transpose then yields ytldT3 rows 3t..3t+2 = that triple.
        ytld3 = pw.tile([P, NJ, 3 * T], F16)
        nc.vector.memset(ytld3, 1.0)
        nc.vector.memset(ytld3[:, :, 1::3], 0.0)
        nc.vector.tensor_scalar_mul(out=ytld3[:, :, 0::3], in0=yb,
                                    scalar1=scal["w_y"])
        for j in range(NJ):
            tpy = psum_t.tile([3 * T, P], F16, name="tpy", tag="pst")
            nc.tensor.transpose(tpy, ytld3[:, j, :], ident)
            nc.scalar.copy(out=ytldT3[:, j * P:(j + 1) * P], in_=tpy)

        _warm_on(yb[:, 0, :], yb, 1)
        _wfill(3)

        with tc.tile_pool(name="penc", bufs=1) as penc:
            c1 = penc.tile([P, KE, BS], F32)
            c2 = penc.tile([P, KE, BS], F32)
            xtA1 = [penc.tile([P, BS], F16, name=f"xa1_{t}", tag=f"xa1_{t}")
                    for t in range(T)]
            xtA2 = [penc.tile([P, BS], F16, name=f"xa2_{t}", tag=f"xa2_{t}")
                    for t in range(T)]
            xt1 = penc.tile([P, NJ, T, 17], F16)
            xt2 = penc.tile([P, NJ, T, 16], F16)

            def _xtile_transpose(t, evac_engine):
                # 4x col-tiled concurrent transposes into partition strips
                # 0/32/64/96 of one PSUM tile, evacuated with a single
                # full-width copy: row-strip replicas for the 4x row-tiled
                # input matmuls with no SBUF->SBUF DMA storm.
                for j in range(NJ):
                    tp1 = psum_g.tile([P, P], F16, name="tp1", tag="psg")
                    for g in range(4):
                        nc.tensor.transpose(tp1[32 * g:32 * g + 17, :],
                                            xt1[:, j, t, :], ident,
                                            tile_position=(0, 32 * g))
                    if evac_engine == "split":
                        nc.scalar.copy(out=xtA1[t][:, j * P:(j + 1) * P],
                                       in_=tp1)
                    else:
                        nc.vector.tensor_copy(
                            out=xtA1[t][:, j * P:(j + 1) * P], in_=tp1)
                    tp2 = psum_g.tile([P, P], F16, name="tp2", tag="psg")
                    for g in range(4):
                        nc.tensor.transpose(tp2[32 * g:32 * g + 16, :],
                                            xt2[:, j, t, :], ident,
                                            tile_position=(0, 32 * g))
                    nc.vector.tensor_copy(out=xtA2[t][:, j * P:(j + 1) * P],
                                          in_=tp2)

            with tc.tile_pool(name="ptmp", bufs=1) as ptmp:
                # ---------------- encoder attention precompute ------------
                xyb = ptmp.tile([P, NJ, T, 16], F32)
                nc.vector.tensor_copy(out=xyb[:, :, :, 0:IN2], in_=xb0)
                nc.vector.tensor_copy(out=xyb[:, :, :, IN2], in_=yb[:, :, :])
                _warm_on(xb0[:, 0, 0, :], xb0[:, 1, :, :], 1)
                _wfill(3)
                mmb = ptmp.tile([P, NJ, T, IN2], F32)
                nc.vector.tensor_mul(
                    out=mmb,
                    in0=xyb[:, :, :, 0:IN2],
                    in1=yb.unsqueeze(3).to_broadcast([P, NJ, T, IN2]),
                )
                # Wf-weighted sums over t in 3 independent sub-chains per
                # base so the DVE pipeline isn't one long dependency chain.
                bases = []
                for bi, src_ in ((0, xyb), (1, mmb)):
                    Fb = 16 if bi == 0 else IN2
                    parts = []
                    for c in range(3):
                        pb = ptmp.tile([P, NJ, Fb], F32, name=f"b{bi}p{c}")
                        nc.vector.tensor_scalar_mul(out=pb,
                                                    in0=src_[:, :, 3 * c, :],
                                                    scalar1=Wf[3 * c])
                        for t in (3 * c + 1, 3 * c + 2):
                            nc.vector.scalar_tensor_tensor(
                                out=pb, in0=src_[:, :, t, :], scalar=Wf[t],
                                in1=pb, op0=ALU.mult, op1=ALU.add)
                        parts.append(pb)
                    nc.vector.tensor_add(out=parts[0], in0=parts[0], in1=parts[1])
                    nc.vector.tensor_add(out=parts[0], in0=parts[0], in1=parts[2])
                    bases.append(parts[0])
                    if bi == 0:
                        _warm_on(xyb[:, 0, 0, :], xyb[:, 1, :, :], 1)
                        _wfill(3)
                base1, base2 = bases
                _warm_on(mmb[:, 0, 0, :], mmb[:, 1, :, :], 1)
                _wfill(3)

                # softmax via sigmoid identity: e^x = s/(1-s); no Exp table
                # is ever loaded so the Scalar engine never swaps tables.
                a1 = ptmp.tile([P, NJ, 16], F32)
                a2 = ptmp.tile([P, NJ, IN2], F32)
                for bi, (base, a) in enumerate(((base1, a1), (base2, a2))):
                    Fb = 16 if bi == 0 else IN2
                    for j in range(NJ):
                        sg = psm.tile([P, Fb], F32, name="sg",
                                      tag=f"sg{bi}{j}")
                        nc.scalar.activation(out=sg, in_=base[:, j, :],
                                             func=AF.Sigmoid)
                        om = psm.tile([P, Fb], F32, name="om",
                                      tag=f"om{bi}{j}")
                        nc.vector.tensor_scalar_mul(out=om, in0=sg,
                                                    scalar1=-1.0)
                        nc.vector.tensor_scalar_add(out=om, in0=om,
                                                    scalar1=1.0)
                        nc.vector.reciprocal(out=om, in_=om)
                        nc.vector.tensor_mul(out=a[:, j, :], in0=sg, in1=om)
                        ssum = psm.tile([P, 1], F32, name="ssum",
                                        tag=f"ssum{bi}{j}")
                        nc.vector.reduce_sum(out=ssum, in_=a[:, j, :],
                                             axis=mybir.AxisListType.X)
                        inv = psm.tile([P, 1], F32, name="inv", tag=f"inv{bi}{j}")
                        nc.vector.reciprocal(out=inv, in_=ssum)
                        nc.vector.tensor_scalar_mul(out=a[:, j, :], in0=a[:, j, :],
                                                    scalar1=inv)

                _warm_on(a1[:, 0, :], a1[:, 1, :], 1)
                _wfill(3)
                # last column = 1.0 so the transpose yields the ones row that
                # carries the bias through the K-augmented matmul
                nc.vector.memset(xt1[:, :, :, 16:17], 1.0)
                nc.vector.tensor_mul(
                    out=xt1[:, :, :, 0:16], in0=xyb,
                    in1=a1.unsqueeze(2).to_broadcast([P, NJ, T, 16]))
                nc.vector.memset(xt2[:, :, :, IN2:16], 1.0)
                nc.vector.tensor_mul(
                    out=xt2[:, :, :, 0:IN2], in0=mmb,
                    in1=a2.unsqueeze(2).to_broadcast([P, NJ, T, IN2]))

                _warm_on(ident, xt1[:, 0, :, :], 1)
                _wfill(3)
                if upto == "pre":
                    nc.vector.tensor_copy(out=osb, in_=xt1[:, :, 0, 0:1])

            # first T_PRE steps' transposes before the recurrence; the rest
            # are interleaved into the recurrence loop (step t emits t+T_PRE)
            for t in range(T_PRE):
                _xtile_transpose(t, "split")
                if t < 4:
                    _wfill(2)

            # staged decoder-input rows: [w_y*y_t; 0; 1] into the 4 strips
            # (overlaps the encoder; the A row lands post-encoder)
            for t in range(T):
                for g in range(4):
                    eng = nc.sync if (t * 4 + g) % 2 == 0 else nc.gpsimd
                    eng.dma_start(out=ytA[t][32 * g:32 * g + 3, :],
                                  in_=ytldT3[3 * t:3 * t + 3, :])

            # ---------------- encoder recurrence + score matvecs ----------
            # States in hidden-chunk pairs [P, 2, BS] fp16: rhs layout for
            # the hidden matmuls and lhsT for the score matvecs.
            with tc.tile_pool(name="px16", bufs=3) as px16, \
                 tc.tile_pool(name="pg", bufs=5) as pg:
                prev16 = None
                for t in range(T if upto != "pre" else 0):
                    if t + T_PRE < T:
                        _xtile_transpose(t + T_PRE, "vector")
                    # HAM: t=0 is input-only (choppy waves) and t=0/t=1
                    # cell math leaves the PE idle; warm bursts keep the
                    # clock gate at 8/8 so t=1/t=2 stream at 2.4 GHz.
                    if t == 1:
                        _wfill(14)
                    elif t == 2:
                        _wfill(6)
                    xe16 = [px16.tile([P, 2, BS], F16, name=f"xe16_{i}",
                                      tag=f"xe16_{i}") for i in range(4)]
                    for br, (wiaS, nk, whhX, cbr) in enumerate((
                            (wiaS1, 17, whh1, c1),
                            (wiaS2, 16, whh2, c2))):
                        xtA = xtA1[t] if br == 0 else xtA2[t]
                        for kp in range(KE // 2):
                            pss = [psum_g.tile([P, 2, BS], F32, name=f"ps{g}",
                                               tag="psg") for g in range(4)]
                            # Per half: 4x concurrent row-tiled input
                            # matmuls, then the fp8-weight x fp16-state
                            # accumulates.  Each half's chain completes
                            # before the next half's start=True (a start
                            # clears has_written for the whole bank).
                            for half in range(2):
                                for g in range(4):
                                    m = g * KE + 2 * kp + half
                                    nc.tensor.matmul(
                                        pss[g][:, half, :],
                                        wiaS[32 * g:32 * g + nk,
                                             m * P:(m + 1) * P],
                                        xtA[32 * g:32 * g + nk, :],
                                        start=True, stop=(t == 0),
                                        tile_position=(32 * g, 0))
                                if t > 0:
                                    for g in range(4):
                                        m = g * KE + 2 * kp + half
                                        for k in range(KE):
                                            nc.tensor.matmul(
                                                pss[g][:, half, :],
                                                whhX[:, k, m * P:(m + 1) * P],
                                                prev16[2 * br + k // 2][:, k % 2, :],
                                                start=False, stop=(k == KE - 1))
                            if t == 0:
                                _wfill(2)
                            gt = pg.tile([P, 4, 2, BS], F16, name="gt", tag="ge")
                            for g in range(4):
                                fn = AF.Tanh if g == 2 else AF.Sigmoid
                                nc.scalar.activation(out=gt[:, g, :, :],
                                                     in_=pss[g], func=fn)
                            cs = cbr[:, 2 * kp:2 * kp + 2, :]
                            if t == 0:
                                nc.vector.tensor_mul(out=cs, in0=gt[:, 0, :, :],
                                                     in1=gt[:, 2, :, :])
                            else:
                                u = pu.tile([P, 2, BS], F32, name="u", tag="u")
                                nc.vector.tensor_mul(out=u, in0=gt[:, 0, :, :],
                                                     in1=gt[:, 2, :, :])
                                nc.vector.tensor_mul(out=cs, in0=gt[:, 1, :, :],
                                                     in1=cs)
                                nc.vector.tensor_add(out=cs, in0=cs, in1=u)
                            nc.scalar.activation(out=gt[:, 2, :, :], in_=cs,
                                                 func=AF.Tanh)
                            xi = 2 * br + kp
                            nc.vector.tensor_mul(out=xe16[xi],
                                                 in0=gt[:, 3, :, :],
                                                 in1=gt[:, 2, :, :])
                    # score/q/r matvecs against the 3 packed columns; the
                    # score column goes through sigmoid NOW so the softmax
                    # after the encoder is pure Vector math.
                    for j in range(NJ):
                        psq = psum_q.tile([P, 3], F32, name="psq", tag="psq")
                        for k in range(KD):
                            nc.tensor.matmul(psq,
                                             xe16[k // 2][:, k % 2,
                                                          j * P:(j + 1) * P],
                                             v3t[:, k, :],
                                             start=(k == 0), stop=(k == KD - 1))
                        nc.scalar.activation(out=es[:, j, t:t + 1],
                                             in_=psq[:, 0:1], func=AF.Sigmoid)
                        nc.vector.tensor_copy(out=sq2[:, j, t, :],
                                              in_=psq[:, 1:3])
                    prev16 = xe16

            # ------------- decoder attention (post-encoder, ~2us) ---------
            # keep the PE busy right at the start of the gap
            if upto in ("beta", "dec", "full"):
                _wfill(8)
                _warm_on(ident[0:3 * T, :], ytldT3, 1)
            if upto == "enc":
                nc.vector.tensor_copy(out=osb, in_=es[:, :, 0:1])
                dbgb = pw.tile([P, NJ, T, 3], F32, name="dbgb")
                nc.vector.tensor_copy(out=dbgb[:, :, :, 0], in_=es)
                nc.vector.tensor_copy(out=dbgb[:, :, :, 1:3], in_=sq2)
                nc.sync.dma_start(out=dbgd.rearrange("(j p) t c -> p j t c", p=P),
                                  in_=dbgb)
            for j in range(NJ if upto in ("beta", "dec", "full") else 0):
                # e_t = s/(1-s); beta-weighted sums of q (A) and r (ctxw)
                e = psm.tile([P, T], F32, name="e", tag=f"e{j}")
                om = psm.tile([P, T], F32, name="omb", tag=f"omb{j}")
                nc.vector.tensor_scalar_mul(out=om, in0=es[:, j, :],
                                            scalar1=-1.0)
                nc.vector.tensor_scalar_add(out=om, in0=om, scalar1=1.0)
                nc.vector.reciprocal(out=om, in_=om)
                _warm_on(om, om, 1)
                _wfill(2)
                nc.vector.tensor_mul(out=e, in0=es[:, j, :], in1=om)
                ssum = psm.tile([P, 1], F32, name="ssum", tag=f"bsum{j}")
                nc.vector.reduce_sum(out=ssum, in_=e, axis=mybir.AxisListType.X)
                _warm_on(e, e, 1)
                _wfill(2)
                inv = psm.tile([P, 1], F32, name="inv", tag=f"binv{j}")
                nc.vector.reciprocal(out=inv, in_=ssum)
                tmp9 = psm.tile([P, T], F32, name="tmp9", tag=f"tmp9{j}")
                eq = psm.tile([P, 1], F32, name="eq", tag=f"eq{j}")
                nc.vector.tensor_mul(out=tmp9, in0=e, in1=sq2[:, j, :, 0])
                nc.vector.reduce_sum(out=eq, in_=tmp9, axis=mybir.AxisListType.X)
                nc.vector.tensor_scalar_mul(out=A_t[:, j:j + 1], in0=eq,
                                            scalar1=inv)
                tmp9b = psm.tile([P, T], F32, name="tmp9b", tag=f"tmp9b{j}")
                er = psm.tile([P, 1], F32, name="er", tag=f"er{j}")
                nc.vector.tensor_mul(out=tmp9b, in0=e, in1=sq2[:, j, :, 1])
                nc.vector.reduce_sum(out=er, in_=tmp9b, axis=mybir.AxisListType.X)
                nc.vector.tensor_scalar_mul(out=er, in0=er, scalar1=inv)
                # ctxw = er + fcf_b  (bias folded so the tail is one op)
                nc.vector.tensor_scalar_add(out=ctxw[:, j:j + 1], in0=er,
                                            scalar1=scal["fcf_b"])

            # A row -> the 4 strips of every step tile (36 x 512B DMAs,
            # t-ascending and split across two idle queues, so decoder t=0
            # starts ~1us after A is known)
            if upto in ("dec", "full"):
                nc.vector.tensor_copy(out=A16, in_=A_t)
                arps = psum_q.tile([1, BS], F16, name="arps", tag="psq")
                for j in range(NJ):
                    nc.tensor.transpose(arps[:, j * P:(j + 1) * P],
                                        A16[:, j:j + 1], ident)
                nc.scalar.copy(out=arowT, in_=arps)
                _warm_on(ident[0:1, :], arowT, 1)
                _wfill(2)
                for t in range(T):
                    for g in range(4):
                        eng = nc.sync if (t * 4 + g) % 2 == 0 else nc.gpsimd
                        eng.dma_start(
                            out=ytA[t][32 * g + 1:32 * g + 2, :],
                            in_=arowT)

        if upto == "beta":
            nc.vector.tensor_copy(out=osb, in_=A_t.unsqueeze(2))

        # ---------------- decoder recurrence ----------------
        ndec = T if upto in ("dec", "full") else 0
        dT16p = None
        pdt16 = ctx.enter_context(tc.tile_pool(name="pdt16", bufs=2))
        with tc.tile_pool(name="pgd", bufs=5) as pgd:
            for t in range(ndec):
                if t == 1:
                    _wfill(14)
                elif t == 2:
                    _wfill(6)
                dprev16 = dT16p
                dT16p = [pdt16.tile([P, 2, BS], F16, name=f"dT16_{i}",
                                    tag=f"dT16_{i}") for i in range(KD // 2)]
                for kp in range(KD // 2):
                    pss = [psum_g.tile([P, 2, BS], F32, name=f"psd{g}",
                                       tag="psg") for g in range(4)]
                    for half in range(2):
                        for g in range(4):
                            m = g * KD + 2 * kp + half
                            nc.tensor.matmul(
                                pss[g][:, half, :],
                                wid3R[32 * g:32 * g + 3, m * P:(m + 1) * P],
                                ytA[t][32 * g:32 * g + 3, :],
                                start=True, stop=(t == 0),
                                tile_position=(32 * g, 0))
                        if t > 0:
                            for g in range(4):
                                m = g * KD + 2 * kp + half
                                for k in range(KD):
                                    nc.tensor.matmul(
                                        pss[g][:, half, :],
                                        whhd[:, k, m * P:(m + 1) * P],
                                        dprev16[k // 2][:, k % 2, :],
                                        start=False, stop=(k == KD - 1))
                    if t == 0:
                        _wfill(2)
                    gt = pgd.tile([P, 4, 2, BS], F16, name="gtd", tag="gd")
                    for g in range(4):
                        fn = AF.Tanh if g == 2 else AF.Sigmoid
                        nc.scalar.activation(out=gt[:, g, :, :], in_=pss[g],
                                             func=fn)
                    cs = cd[:, 2 * kp:2 * kp + 2, :]
                    if t == 0:
                        nc.vector.tensor_mul(out=cs, in0=gt[:, 0, :, :],
                                             in1=gt[:, 2, :, :])
                    else:
                        u = pu.tile([P, 2, BS], F32, name="ud", tag="u")
                        nc.vector.tensor_mul(out=u, in0=gt[:, 0, :, :],
                                             in1=gt[:, 2, :, :])
                        nc.vector.tensor_mul(out=cs, in0=gt[:, 1, :, :], in1=cs)
                        nc.vector.tensor_add(out=cs, in0=cs, in1=u)
                    nc.scalar.activation(out=gt[:, 2, :, :], in_=cs, func=AF.Tanh)
                    nc.vector.tensor_mul(out=dT16p[kp], in0=gt[:, 3, :, :],
                                         in1=gt[:, 2, :, :])

        # ---------------- output ----------------
        if upto == "dec":
            nc.vector.tensor_copy(out=osb, in_=cd[:, 0:NJ, 0:1])
        if upto == "full":
            for j in range(NJ):
                psf = psum_q.tile([P, 1], F32, name="psf", tag="psq")
                for k in range(KD):
                    nc.tensor.matmul(psf,
                                     dT16p[k // 2][:, k % 2, j * P:(j + 1) * P],
                                     wdt[:, k, :],
                                     start=(k == 0), stop=(k == KD - 1))
                nc.vector.scalar_tensor_tensor(
                    out=osb[:, j, :], in0=psf, scalar=1.0,
                    in1=ctxw[:, j:j + 1], op0=ALU.mult, op1=ALU.add)
        nc.sync.dma_start(out=outd.rearrange("(j p) c -> p j c", p=P), in_=osb)

    nc.compile()
    return nc


def _run(inputs, trace=False, upto="full"):
    weights, scal = _pack_weights(inputs)
    nc = _build(scal, upto=upto)
    X = np.ascontiguousarray(_np(inputs["X"]))
    Y = np.ascontiguousarray(_np(inputs["y_prev"]))
    in_maps = []
    for c in range(NCORES):
        m = dict(weights)
        m["x"] = np.ascontiguousarray(X[c * BS:(c + 1) * BS])
        m["y"] = np.ascontiguousarray(Y[c * BS:(c + 1) * BS])
        in_maps.append(m)
    res = run_bass_kernel_spmd(nc, in_maps, core_ids=list(range(NCORES)), trace=trace)
    out = np.concatenate([np.asarray(res.results[i]["out"]) for i in range(NCORES)],
                         axis=0).astype(np.float32)
    return out, res


def kernel(**inputs):
    out, _ = _run(inputs, trace=False)
    return out


# revision 14
# speedup vs baseline: 1.0009x; 1.0009x over previous
"""DA-RNN (dual-stage attention RNN) forward pass on 8 TRN2 NeuronCores.

Data-parallel: batch 2048 sharded 256 per core, weights replicated.

Algebraic structure exploited (validated against the reference in numpy):
  * Both attention blocks add their state-dependent term as a per-sample
    constant across the softmax axis, so softmax cancels it.  The encoder
    input attention (a1, a2) and the decoder temporal attention (beta) are
    input-only precomputes, and the decoder context vector is constant
    across decoder steps.
  * context only enters through dot products (fc_W, fc_final_W): three
    matvec columns [v, fcW1, w_c] against X_encoded give score/q/r per
    (sample, t); softmax-weighted sums of q, r give the decoder LSTM input
    offset (A) and the output contribution.
  * The decoder LSTM input is scalar per sample.  It is fed as a K=3
    augmented matmul with rhs rows [w_y*y_t; A; 1] against lhsT rows
    [Wd0; Wd0; bd + Wd0*fc_b]: the y rows and ones rows are input-only and
    are staged into the 4 row-tiled strips BEFORE the encoder runs; only
    the single A row (shared by all steps) is written after the encoder,
    so the encoder->decoder transition has a ~2us serial path instead of a
    serialized 36-DMA storm.
  * softmax exp is computed via the sigmoid identity e^x = s/(1-s)
    (scores are O(0.3)), so the Scalar engine never swaps activation
    tables (Exp <-> Sigmoid/Tanh swaps cost 1.3us each on the serial
    path).

Precision plan (validated numerically on the fixed inputs):
  * All hidden (recurrent) weights are fp8e4 (halves weight DMA);
    states stay fp16 and stream as the matmul moving operand (mixed
    fp8-weight x fp16-state matmuls run at full bf16 rate and register as
    activity to the HAM PE clock gate, keeping the array at 2.4 GHz).
  * Input matmuls (K=17/16/3) are packed 4x concurrent via row-tiled
    tile_position strips with replicated weights/rhs.
  * Within a PSUM bank, each accumulation group's chain completes before
    the next group's start=True (a start clears has_written bank-wide).

HAM discipline: the PE clock gate needs sustained activity; dummy/dep-
gated warm matmuls bridge every >3.4us PE-idle window (DMA preamble,
attention precompute, encoder->decoder transition).

On-chip layout: feature-major [dim on partitions (128-chunks), batch on
free].  LSTM states stay in that layout so no transposes in the
recurrences.  PSUM accumulates fp32 throughout.
"""

import sys
import os

sys.path.insert(0, "/opt/trn_rl_repo")
os.environ.setdefault("MYCRO_LOCAL_CACHE", "1")

from contextlib import ExitStack

import numpy as np
import ml_dtypes

import concourse.bass as bass
import concourse.mybir as mybir
import concourse.tile as tile
from concourse import bacc
from concourse.bass_utils import run_bass_kernel_spmd
from concourse.masks import make_identity

F32 = mybir.dt.float32
F16 = mybir.dt.float16
F8 = mybir.dt.float8e4
AF = mybir.ActivationFunctionType
ALU = mybir.AluOpType

NCORES = 8
P = 128
BS = 256          # batch per core
NJ = 2            # 128-partition batch chunks
T = 9             # recurrence steps (T-1 in the reference)
H = 512
IN2 = 15
ME = 16           # encoder gate chunks (4H/128)
KE = 4            # encoder hidden chunks (H/128)
MD = 32           # decoder gate chunks (8H/128)
KD = 8            # decoder hidden chunks (2H/128)

N_WARM_PREFIX = 12
T_PRE = T         # x-tilde transpose steps done before the recurrence


def _np(a):
    return np.asarray(a, dtype=np.float32)


def _f16(a):
    return np.ascontiguousarray(np.asarray(a, dtype=np.float32).astype(np.float16))


def _f8(a):
    return np.ascontiguousarray(
        np.asarray(a, dtype=np.float32).astype(ml_dtypes.float8_e4m3))


def _pack_weights(inp):
    """Host-side weight folding (weight-only transforms; no input math)."""
    Wih1, Whh1 = _np(inp["enc_lstm_Wih"]), _np(inp["enc_lstm_Whh"])
    b1 = _np(inp["enc_lstm_bih"]) + _np(inp["enc_lstm_bhh"])
    Wih2, Whh2 = _np(inp["enc_lstm1_Wih"]), _np(inp["enc_lstm1_Whh"])
    b2 = _np(inp["enc_lstm1_bih"]) + _np(inp["enc_lstm1_bhh"])
    Wd_ih, Wd_hh = _np(inp["dec_lstm_Wih"]), _np(inp["dec_lstm_Whh"])
    bd = _np(inp["dec_lstm_bih"]) + _np(inp["dec_lstm_bhh"])
    attn1_W = _np(inp["dec_attn1_W"])
    attn2_w = _np(inp["dec_attn2_W"])[0]
    fc_W = _np(inp["fc_W"])[0]
    fcf_W = _np(inp["fc_final_W"])[0]

    W1x = attn1_W[:, 4 * H:]                        # (512, 1024)
    v = W1x.T @ attn2_w                             # (1024,)
    fcW1 = fc_W[:2 * H]
    w_c = fcf_W[2 * H:]
    w_d = fcf_W[:2 * H]

    # input+bias weights, replicated at 4 row strips for row-tiled quads
    wia1 = np.concatenate([Wih1.T, b1[None, :]], axis=0)            # (17, 2048)
    wia2 = np.concatenate([Wih2.T, b2[None, :]], axis=0)            # (16, 2048)
    Wd0 = Wd_ih[:, 0]
    wid3 = np.stack([Wd0, Wd0,
                     bd + Wd0 * float(_np(inp["fc_b"])[0])],
                    axis=0)                                         # (3, 4096)

    def _part(w, k):  # (k*P, m) -> (P, k, m) partition-contiguous
        m = w.shape[1]
        return np.ascontiguousarray(
            w.reshape(k, P, m).transpose(1, 0, 2))

    def _strips(w):  # (nk, m) -> (P, m) replicated at partition strips 32g
        nk, m = w.shape
        out = np.zeros((P, m), np.float32)
        for g in range(4):
            out[32 * g:32 * g + nk] = w
        return out

    weights = {
        "wia1": _f16(_strips(wia1)),
        "wia2": _f16(_strips(wia2)),
        "wid3": _f16(_strips(wid3)),
        "whh1": _f8(_part(Whh1.T, KE)),                             # (P, KE, 2048)
        "whh2": _f8(_part(Whh2.T, KE)),                             # (P, KE, 2048)
        "whhd": _f8(_part(Wd_hh.T, KD)),                            # (P, KD, 4096)
        "v3": _f16(_part(np.stack([v, fcW1, w_c], axis=1), KD)),    # (P, KD, 3)
        "wd": _f16(_part(w_d[:, None], KD)),                        # (P, KD, 1)
    }
    scalars = {
        "Wf": [float(x) for x in _np(inp["enc_attn_W"])[0, 2 * H:]],    # 9 floats
        "w_y": float(fc_W[2 * H]),
        "fcf_b": float(_np(inp["fc_final_b"])[0]),
    }
    return weights, scalars


def _build(scal, upto="full"):
    nc = bacc.Bacc()

    xd = nc.declare_dram_parameter("x", [BS, T, IN2], F32, isOutput=False)
    yd = nc.declare_dram_parameter("y", [BS, T], F32, isOutput=False)
    wia1d = nc.declare_dram_parameter("wia1", [P, 4 * H], F16, isOutput=False)
    wia2d = nc.declare_dram_parameter("wia2", [P, 4 * H], F16, isOutput=False)
    wid3d = nc.declare_dram_parameter("wid3", [P, 8 * H], F16, isOutput=False)
    whh1d = nc.declare_dram_parameter("whh1", [P, KE, 4 * H], F8, isOutput=False)
    whh2d = nc.declare_dram_parameter("whh2", [P, KE, 4 * H], F8, isOutput=False)
    whhdd = nc.declare_dram_parameter("whhd", [P, KD, 8 * H], F8, isOutput=False)
    v3d = nc.declare_dram_parameter("v3", [P, KD, 3], F16, isOutput=False)
    wdd = nc.declare_dram_parameter("wd", [P, KD, 1], F16, isOutput=False)
    outd = nc.declare_dram_parameter("out", [BS, 1], F32, isOutput=True)
    dbgd = (nc.declare_dram_parameter("dbg", [BS, T, 3], F32, isOutput=True)
            if upto == "enc" else None)

    Wf = scal["Wf"]

    with ExitStack() as ctx:
        tc = ctx.enter_context(tile.TileContext(nc))
        # persistent pools
        pw = ctx.enter_context(tc.tile_pool(name="pw", bufs=1))
        psm = ctx.enter_context(tc.tile_pool(name="psm", bufs=4))     # small f32 scratch
        pu = ctx.enter_context(tc.tile_pool(name="pu", bufs=4))       # cell temp
        pya = ctx.enter_context(tc.tile_pool(name="pya", bufs=1))
        psum_g = ctx.enter_context(tc.tile_pool(name="psum_g", bufs=5, space="PSUM"))
        psum_t = ctx.enter_context(tc.tile_pool(name="psum_t", bufs=1, space="PSUM"))
        psum_q = ctx.enter_context(tc.tile_pool(name="psum_q", bufs=2, space="PSUM"))

        # ---------------- input DMAs (critical path first) ----------------
        yb = pw.tile([P, NJ, T], F32)
        nc.sync.dma_start(out=yb, in_=yd.rearrange("(j p) t -> p j t", p=P))
        xb0 = pw.tile([P, NJ, T, IN2], F32)
        xd_r = xd.rearrange("(j p) t f -> p j t f", p=P)
        for j in range(NJ):
            nc.sync.dma_start(out=xb0[:, j, :, :], in_=xd_r[:, j, :, :])
        wiaS1 = pw.tile([P, 4 * H], F16)
        nc.sync.dma_start(out=wiaS1, in_=wia1d[:, :])
        wiaS2 = pw.tile([P, 4 * H], F16)
        nc.sync.dma_start(out=wiaS2, in_=wia2d[:, :])
        v3t = pw.tile([P, KD, 3], F16)
        nc.sync.dma_start(out=v3t, in_=v3d[:, :, :])
        wdt = pw.tile([P, KD, 1], F16)
        nc.sync.dma_start(out=wdt, in_=wdd[:, :, :])

        # identity/filler tiles BEFORE the gpsimd DMA block: make_identity
        # runs on gpsimd, and the PE warm-up fills need it immediately
        ident = pw.tile([P, P], F16)
        make_identity(nc, ident)
        wbig = pw.tile([P, 480], F16)
        nc.vector.memset(wbig, 1.0)

        # hidden weights on the gpsimd queue: whh1/whh2 first (needed at
        # encoder t=1), then the decoder weights (needed ~200us later)
        whh1 = pw.tile([P, KE, 4 * H], F8)
        nc.gpsimd.dma_start(out=whh1, in_=whh1d[:, :, :])
        whh2 = pw.tile([P, KE, 4 * H], F8)
        nc.gpsimd.dma_start(out=whh2, in_=whh2d[:, :, :])
        wid3R = pw.tile([P, 8 * H], F16)
        nc.gpsimd.dma_start(out=wid3R, in_=wid3d[:, :])
        whhd = pw.tile([P, KD, 8 * H], F8)
        nc.gpsimd.dma_start(out=whhd, in_=whhdd[:, :, :])

        # PE warm-up helpers.  The HAM clock gate needs sustained activity;
        # _warm is un-gated (runs immediately at its queue position),
        # _warm_on is gated on a data tile so it fires mid-pipeline.
        def _warm(n):
            for _ in range(n):
                wps = psum_q.tile([P, P], F32, name="wps", tag="psq")
                nc.tensor.matmul(wps, ident, ident, start=True, stop=True)

        def _wfill(n):
            # dense filler: 480-col streams keep the HAM activity monitor
            # fed (un-throttle needs ~27us of SUSTAINED busy on this part,
            # so every idle window must be prevented, not repaired)
            for _ in range(n):
                wps = psum_q.tile([P, 480], F32, name="wf", tag="psq")
                nc.tensor.matmul(wps, ident, wbig, start=True, stop=True)

        wseq = [0]

        def _warm_on(lhs, rhs, n=1):
            m = lhs.free_size()
            nfree = rhs.free_size()
            for _ in range(n):
                wseq[0] += 1
                wps = psum_q.tile([min(m, P), min(nfree, 480)], F32,
                                  name=f"wo{wseq[0]}", tag="psq")
                nc.tensor.matmul(wps, lhs, rhs, start=True, stop=True)

        _wfill(6)

        # persistent state / small tiles
        sq2 = pw.tile([P, NJ, T, 2], F32)     # q, r matvec results
        es = pw.tile([P, NJ, T], F32)         # sigmoid(score) per step
        A_t = pw.tile([P, NJ], F32)
        A16 = pw.tile([P, NJ], F16)
        ctxw = pw.tile([P, NJ], F32)
        ytldT3 = pw.tile([3 * T, BS], F16)    # [w_y*y_t; 0; 1] transposed
        arowT = pw.tile([1, BS], F16)         # A row, post-encoder
        cd = pw.tile([P, KD, BS], F32)
        osb = pw.tile([P, NJ, 1], F32)
        ytA = [pya.tile([P, BS], F16, name=f"ytA{t}", tag=f"ytA{t}")
               for t in range(T)]

        # ---------------- decoder input rows (pre-encoder part) -----------
        # ytld3 free layout f = 3t + r with rows [w_y*y_t; A=0; 1]; the
        # transpose then yields ytldT3 rows 3t..3t+2 = that triple.
        ytld3 = pw.tile([P, NJ, 3 * T], F16)
        nc.vector.memset(ytld3, 1.0)
        nc.vector.memset(ytld3[:, :, 1::3], 0.0)
        nc.vector.tensor_scalar_mul(out=ytld3[:, :, 0::3], in0=yb,
                                    scalar1=scal["w_y"])
        for j in range(NJ):
            tpy = psum_t.tile([3 * T, P], F16, name="tpy", tag="pst")
            nc.tensor.transpose(tpy, ytld3[:, j, :], ident)
            nc.scalar.copy(out=ytldT3[:, j * P:(j + 1) * P], in_=tpy)

        _warm_on(yb[:, 0, :], yb, 1)
        _wfill(3)

        with tc.tile_pool(name="penc", bufs=1) as penc:
            c1 = penc.tile([P, KE, BS], F32)
            c2 = penc.tile([P, KE, BS], F32)
            xtA1 = [penc.tile([P, BS], F16, name=f"xa1_{t}", tag=f"xa1_{t}")
                    for t in range(T)]
            xtA2 = [penc.tile([P, BS], F16, name=f"xa2_{t}", tag=f"xa2_{t}")
                    for t in range(T)]
            xt1 = penc.tile([P, NJ, T, 17], F16)
            xt2 = penc.tile([P, NJ, T, 16], F16)

            def _xtile_transpose(t, evac_engine):
                # 4x col-tiled concurrent transposes into partition strips
                # 0/32/64/96 of one PSUM tile, evacuated with a single
                # full-width copy: row-strip replicas for the 4x row-tiled
                # input matmuls with no SBUF->SBUF DMA storm.
                for j in range(NJ):
                    tp1 = psum_g.tile([P, P], F16, name="tp1", tag="psg")
                    for g in range(4):
                        nc.tensor.transpose(tp1[32 * g:32 * g + 17, :],
                                            xt1[:, j, t, :], ident,
                                            tile_position=(0, 32 * g))
                    if evac_engine == "split":
                        nc.scalar.copy(out=xtA1[t][:, j * P:(j + 1) * P],
                                       in_=tp1)
                    else:
                        nc.vector.tensor_copy(
                            out=xtA1[t][:, j * P:(j + 1) * P], in_=tp1)
                    tp2 = psum_g.tile([P, P], F16, name="tp2", tag="psg")
                    for g in range(4):
                        nc.tensor.transpose(tp2[32 * g:32 * g + 16, :],
                                            xt2[:, j, t, :], ident,
                                            tile_position=(0, 32 * g))
                    nc.vector.tensor_copy(out=xtA2[t][:, j * P:(j + 1) * P],
                                          in_=tp2)

            with tc.tile_pool(name="ptmp", bufs=1) as ptmp:
                # ---------------- encoder attention precompute ------------
                xyb = ptmp.tile([P, NJ, T, 16], F32)
                nc.vector.tensor_copy(out=xyb[:, :, :, 0:IN2], in_=xb0)
                nc.vector.tensor_copy(out=xyb[:, :, :, IN2], in_=yb[:, :, :])
                _warm_on(xb0[:, 0, 0, :], xb0[:, 1, :, :], 1)
                _wfill(3)
                mmb = ptmp.tile([P, NJ, T, IN2], F32)
                nc.vector.tensor_mul(
                    out=mmb,
                    in0=xyb[:, :, :, 0:IN2],
                    in1=yb.unsqueeze(3).to_broadcast([P, NJ, T, IN2]),
                )
                # Wf-weighted sums over t in 3 independent sub-chains per
                # base so the DVE pipeline isn't one long dependency chain.
                bases = []
                for bi, src_ in ((0, xyb), (1, mmb)):
                    Fb = 16 if bi == 0 else IN2
                    parts = []
                    for c in range(3):
                        pb = ptmp.tile([P, NJ, Fb], F32, name=f"b{bi}p{c}")
                        nc.vector.tensor_scalar_mul(out=pb,
                                                    in0=src_[:, :, 3 * c, :],
                                                    scalar1=Wf[3 * c])
                        for t in (3 * c + 1, 3 * c + 2):
                            nc.vector.scalar_tensor_tensor(
                                out=pb, in0=src_[:, :, t, :], scalar=Wf[t],
                                in1=pb, op0=ALU.mult, op1=ALU.add)
                        parts.append(pb)
                    nc.vector.tensor_add(out=parts[0], in0=parts[0], in1=parts[1])
                    nc.vector.tensor_add(out=parts[0], in0=parts[0], in1=parts[2])
                    bases.append(parts[0])
                    if bi == 0:
                        _warm_on(xyb[:, 0, 0, :], xyb[:, 1, :, :], 1)
                        _wfill(3)
                base1, base2 = bases
                _warm_on(mmb[:, 0, 0, :], mmb[:, 1, :, :], 1)
                _wfill(3)

                # softmax via sigmoid identity: e^x = s/(1-s); no Exp table
                # is ever loaded so the Scalar engine never swaps tables.
                a1 = ptmp.tile([P, NJ, 16], F32)
                a2 = ptmp.tile([P, NJ, IN2], F32)
                for bi, (base, a) in enumerate(((base1, a1), (base2, a2))):
                    Fb = 16 if bi == 0 else IN2
                    for j in range(NJ):
                        sg = psm.tile([P, Fb], F32, name="sg",
                                      tag=f"sg{bi}{j}")
                        nc.scalar.activation(out=sg, in_=base[:, j, :],
                                             func=AF.Sigmoid)
                        om = psm.tile([P, Fb], F32, name="om",
                                      tag=f"om{bi}{j}")
                        nc.vector.tensor_scalar_mul(out=om, in0=sg,
                                                    scalar1=-1.0)
                        nc.vector.tensor_scalar_add(out=om, in0=om,
                                                    scalar1=1.0)
                        nc.vector.reciprocal(out=om, in_=om)
                        nc.vector.tensor_mul(out=a[:, j, :], in0=sg, in1=om)
                        ssum = psm.tile([P, 1], F32, name="ssum",
                                        tag=f"ssum{bi}{j}")
                        nc.vector.reduce_sum(out=ssum, in_=a[:, j, :],
                                             axis=mybir.AxisListType.X)
                        inv = psm.tile([P, 1], F32, name="inv", tag=f"inv{bi}{j}")
                        nc.vector.reciprocal(out=inv, in_=ssum)
                        nc.vector.tensor_scalar_mul(out=a[:, j, :], in0=a[:, j, :],
                                                    scalar1=inv)

                _warm_on(a1[:, 0, :], a1[:, 1, :], 1)
                _wfill(3)
                # last column = 1.0 so the transpose yields the ones row that
                # carries the bias through the K-augmented matmul
                nc.vector.memset(xt1[:, :, :, 16:17], 1.0)
                nc.vector.tensor_mul(
                    out=xt1[:, :, :, 0:16], in0=xyb,
                    in1=a1.unsqueeze(2).to_broadcast([P, NJ, T, 16]))
                nc.vector.memset(xt2[:, :, :, IN2:16], 1.0)
                nc.vector.tensor_mul(
                    out=xt2[:, :, :, 0:IN2], in0=mmb,
                    in1=a2.unsqueeze(2).to_broadcast([P, NJ, T, IN2]))

                _warm_on(ident, xt1[:, 0, :, :], 1)
                _wfill(3)
                if upto == "pre":
                    nc.vector.tensor_copy(out=osb, in_=xt1[:, :, 0, 0:1])

            # first T_PRE steps' transposes before the recurrence; the rest
            # are interleaved into the recurrence loop (step t emits t+T_PRE)
            for t in range(T_PRE):
                _xtile_transpose(t, "split")
                if t < 4:
                    _wfill(2)

            # staged decoder-input rows: [w_y*y_t; 0; 1] into the 4 strips
            # (overlaps the encoder; the A row lands post-encoder)
            for t in range(T):
                for g in range(4):
                    eng = nc.sync if (t * 4 + g) % 2 == 0 else nc.gpsimd
                    eng.dma_start(out=ytA[t][32 * g:32 * g + 3, :],
                                  in_=ytldT3[3 * t:3 * t + 3, :])

            # ---------------- encoder recurrence + score matvecs ----------
            # States in hidden-chunk pairs [P, 2, BS] fp16: rhs layout for
            # the hidden matmuls and lhsT for the score matvecs.
            with tc.tile_pool(name="px16", bufs=3) as px16, \
                 tc.tile_pool(name="pg", bufs=5) as pg:
                prev16 = None
                for t in range(T if upto != "pre" else 0):
                    if t + T_PRE < T:
                        _xtile_transpose(t + T_PRE, "vector")
                    # HAM: t=0 is input-only (choppy waves) and t=0/t=1
                    # cell math leaves the PE idle; warm bursts keep the
                    # clock gate at 8/8 so t=1/t=2 stream at 2.4 GHz.
                    if t == 1:
                        _wfill(14)
                    elif t == 2:
                        _wfill(6)
                    xe16 = [px16.tile([P, 2, BS], F16, name=f"xe16_{i}",
                                      tag=f"xe16_{i}") for i in range(4)]
                    for br, (wiaS, nk, whhX, cbr) in enumerate((
                            (wiaS1, 17, whh1, c1),
                            (wiaS2, 16, whh2, c2))):
                        xtA = xtA1[t] if br == 0 else xtA2[t]
                        for kp in range(KE // 2):
                            pss = [psum_g.tile([P, 2, BS], F32, name=f"ps{g}",
                                               tag="psg") for g in range(4)]
                            # Per half: 4x concurrent row-tiled input
                            # matmuls, then the fp8-weight x fp16-state
                            # accumulates.  Each half's chain completes
                            # before the next half's start=True (a start
                            # clears has_written for the whole bank).
                            for half in range(2):
                                for g in range(4):
                                    m = g * KE + 2 * kp + half
                                    nc.tensor.matmul(
                                        pss[g][:, half, :],
                                        wiaS[32 * g:32 * g + nk,
                                             m * P:(m + 1) * P],
                                        xtA[32 * g:32 * g + nk, :],
                                        start=True, stop=(t == 0),
                                        tile_position=(32 * g, 0))
                                if t > 0:
                                    for g in range(4):
                                        m = g * KE + 2 * kp + half
                                        for k in range(KE):
                                            nc.tensor.matmul(
                                                pss[g][:, half, :],
                                                whhX[:, k, m * P:(m + 1) * P],
                                                prev16[2 * br + k // 2][:, k % 2, :],
                                                start=False, stop=(k == KE - 1))
                            if t == 0:
                                _wfill(2)
                            gt = pg.tile([P, 4, 2, BS], F16, name="gt", tag="ge")
                            for g in range(4):
                                fn = AF.Tanh if g == 2 else AF.Sigmoid
                                nc.scalar.activation(out=gt[:, g, :, :],
                                                     in_=pss[g], func=fn)
                            cs = cbr[:, 2 * kp:2 * kp + 2, :]
                            if t == 0:
                                nc.vector.tensor_mul(out=cs, in0=gt[:, 0, :, :],
                                                     in1=gt[:, 2, :, :])
                            else:
                                u = pu.tile([P, 2, BS], F32, name="u", tag="u")
                                nc.vector.tensor_mul(out=u, in0=gt[:, 0, :, :],
                                                     in1=gt[:, 2, :, :])
                                nc.vector.tensor_mul(out=cs, in0=gt[:, 1, :, :],
                                                     in1=cs)
                                nc.vector.tensor_add(out=cs, in0=cs, in1=u)
                            nc.scalar.activation(out=gt[:, 2, :, :], in_=cs,
                                                 func=AF.Tanh)
                            xi = 2 * br + kp
                            nc.vector.tensor_mul(out=xe16[xi],
                                                 in0=gt[:, 3, :, :],
                                                 in1=gt[:, 2, :, :])
                    # score/q/r matvecs against the 3 packed columns; the
                    # score column goes through sigmoid NOW so the softmax
                    # after the encoder is pure Vector math.
                    for j in range(NJ):
                        psq = psum_q.tile([P, 3], F32, name="psq", tag="psq")
                        for k in range(KD):
                            nc.tensor.matmul(psq,
                                             xe16[k // 2][:, k % 2,
                                                          j * P:(j + 1) * P],
                                             v3t[:, k, :],
                                             start=(k == 0), stop=(k == KD - 1))
                        nc.scalar.activation(out=es[:, j, t:t + 1],
                                             in_=psq[:, 0:1], func=AF.Sigmoid)
                        nc.vector.tensor_copy(out=sq2[:, j, t, :],
                                              in_=psq[:, 1:3])
                    prev16 = xe16

            # ------------- decoder attention (post-encoder, ~2us) ---------
            # keep the PE busy right at the start of the gap
            if upto in ("beta", "dec", "full"):
                _wfill(8)
                _warm_on(ident[0:3 * T, :], ytldT3, 1)
            if upto == "enc":
                nc.vector.tensor_copy(out=osb, in_=es[:, :, 0:1])
                dbgb = pw.tile([P, NJ, T, 3], F32, name="dbgb")
                nc.vector.tensor_copy(out=dbgb[:, :, :, 0], in_=es)
                nc.vector.tensor_copy(out=dbgb[:, :, :, 1:3], in_=sq2)
                nc.sync.dma_start(out=dbgd.rearrange("(j p) t c -> p j t c", p=P),
                                  in_=dbgb)
            for j in range(NJ if upto in ("beta", "dec", "full") else 0):
                # e_t = s/(1-s); beta-weighted sums of q (A) and r (ctxw)
                e = psm.tile([P, T], F32, name="e", tag=f"e{j}")
                om = psm.tile([P, T], F32, name="omb", tag=f"omb{j}")
                nc.vector.tensor_scalar_mul(out=om, in0=es[:, j, :],
                                            scalar1=-1.0)
                nc.vector.tensor_scalar_add(out=om, in0=om, scalar1=1.0)
                nc.vector.reciprocal(out=om, in_=om)
                _warm_on(om, om, 1)
                _wfill(2)
                nc.vector.tensor_mul(out=e, in0=es[:, j, :], in1=om)
                ssum = psm.tile([P, 1], F32, name="ssum", tag=f"bsum{j}")
                nc.vector.reduce_sum(out=ssum, in_=e, axis=mybir.AxisListType.X)
                _warm_on(e, e, 1)
                _wfill(2)
                inv = psm.tile([P, 1], F32, name="inv", tag=f"binv{j}")
                nc.vector.reciprocal(out=inv, in_=ssum)
                tmp9 = psm.tile([P, T], F32, name="tmp9", tag=f"tmp9{j}")
                eq = psm.tile([P, 1], F32, name="eq", tag=f"eq{j}")
                nc.vector.tensor_mul(out=tmp9, in0=e, in1=sq2[:, j, :, 0])
                nc.vector.reduce_sum(out=eq, in_=tmp9, axis=mybir.AxisListType.X)
                nc.vector.tensor_scalar_mul(out=A_t[:, j:j + 1], in0=eq,
                                            scalar1=inv)
                tmp9b = psm.tile([P, T], F32, name="tmp9b", tag=f"tmp9b{j}")
                er = psm.tile([P, 1], F32, name="er", tag=f"er{j}")
                nc.vector.tensor_mul(out=tmp9b, in0=e, in1=sq2[:, j, :, 1])
                nc.vector.reduce_sum(out=er, in_=tmp9b, axis=mybir.AxisListType.X)
                nc.vector.tensor_scalar_mul(out=er, in0=er, scalar1=inv)
                # ctxw = er + fcf_b  (bias folded so the tail is one op)
                nc.vector.tensor_scalar_add(out=ctxw[:, j:j + 1], in0=er,
                                            scalar1=scal["fcf_b"])

            # A row -> the 4 strips of every step tile (36 x 512B DMAs,
            # t-ascending and split across two idle queues, so decoder t=0
            # starts ~1us after A is known)
            if upto in ("dec", "full"):
                nc.vector.tensor_copy(out=A16, in_=A_t)
                arps = psum_q.tile([1, BS], F16, name="arps", tag="psq")
                for j in range(NJ):
                    nc.tensor.transpose(arps[:, j * P:(j + 1) * P],
                                        A16[:, j:j + 1], ident)
                nc.scalar.copy(out=arowT, in_=arps)
                _warm_on(ident[0:1, :], arowT, 1)
                _wfill(2)
                for t in range(T):
                    for g in range(4):
                        eng = nc.sync if (t * 4 + g) % 2 == 0 else nc.gpsimd
                        eng.dma_start(
                            out=ytA[t][32 * g + 1:32 * g + 2, :],
                            in_=arowT)

        if upto == "beta":
            nc.vector.tensor_copy(out=osb, in_=A_t.unsqueeze(2))

        # ---------------- decoder recurrence ----------------
        ndec = T if upto in ("dec", "full") else 0
        dT16p = None
        pdt16 = ctx.enter_context(tc.tile_pool(name="pdt16", bufs=2))
        with tc.tile_pool(name="pgd", bufs=5) as pgd:
            for t in range(ndec):
                if t == 1:
                    _wfill(14)
                elif t == 2:
                    _wfill(6)
                dprev16 = dT16p
                dT16p = [pdt16.tile([P, 2, BS], F16, name=f"dT16_{i}",
                                    tag=f"dT16_{i}") for i in range(KD // 2)]
                for kp in range(KD // 2):
                    pss = [psum_g.tile([P, 2, BS], F32, name=f"psd{g}",
                                       tag="psg") for g in range(4)]
                    for half in range(2):
                        for g in range(4):
                            m = g * KD + 2 * kp + half
                            nc.tensor.matmul(
                                pss[g][:, half, :],
                                wid3R[32 * g:32 * g + 3, m * P:(m + 1) * P],
                                ytA[t][32 * g:32 * g + 3, :],
                                start=True, stop=(t == 0),
                                tile_position=(32 * g, 0))
                        if t > 0:
                            for g in range(4):
                                m = g * KD + 2 * kp + half
                                for k in range(KD):
                                    nc.tensor.matmul(
                                        pss[g][:, half, :],
                                        whhd[:, k, m * P:(m + 1) * P],
                                        dprev16[k // 2][:, k % 2, :],
                                        start=False, stop=(k == KD - 1))
                    if t == 0:
                        _wfill(2)
                    gt = pgd.tile([P, 4, 2, BS], F16, name="gtd", tag="gd")
                    for g in range(4):
                        fn = AF.Tanh if g == 2 else AF.Sigmoid
                        nc.scalar.activation(out=gt[:, g, :, :], in_=pss[g],
                                             func=fn)
                    cs = cd[:, 2 * kp:2 * kp + 2, :]
                    if t == 0:
                        nc.vector.tensor_mul(out=cs, in0=gt[:, 0, :, :],
                                             in1=gt[:, 2, :, :])
                    else:
                        u = pu.tile([P, 2, BS], F32, name="ud", tag="u")
                        nc.vector.tensor_mul(out=u, in0=gt[:, 0, :, :],
                                             in1=gt[:, 2, :, :])
                        nc.vector.tensor_mul(out=cs, in0=gt[:, 1, :, :], in1=cs)
                        nc.vector.tensor_add(out=cs, in0=cs, in1=u)
                    nc.scalar.activation(out=gt[:, 2, :, :], in_=cs, func=AF.Tanh)
                    nc.vector.tensor_mul(out=dT16p[kp], in0=gt[:, 3, :, :],
                                         in1=gt[:, 2, :, :])

        # ---------------- output ----------------
        if upto == "dec":
            nc.vector.tensor_copy(out=osb, in_=cd[:, 0:NJ, 0:1])
        if upto == "full":
            for j in range(NJ):
                psf = psum_q.tile([P, 1], F32, name="psf", tag="psq")
                for k in range(KD):
                    nc.tensor.matmul(psf,
                                     dT16p[k // 2][:, k % 2, j * P:(j + 1) * P],
                                     wdt[:, k, :],
                                     start=(k == 0), stop=(k == KD - 1))
                nc.vector.scalar_tensor_tensor(
                    out=osb[:, j, :], in0=psf, scalar=1.0,
                    in1=ctxw[:, j:j + 1], op0=ALU.mult, op1=ALU.add)
        nc.sync.dma_start(out=outd.rearrange("(j p) c -> p j c", p=P), in_=osb)

    nc.compile()
    return nc


def _run(inputs, trace=False, upto="full"):
    weights, scal = _pack_weights(inputs)
    nc = _build(scal, upto=upto)
    X = np.ascontiguousarray(_np(inputs["X"]))
    Y = np.ascontiguousarray(_np(inputs["y_prev"]))
    in_maps = []
    for c in range(NCORES):
        m = dict(weights)
        m["x"] = np.ascontiguousarray(X[c * BS:(c + 1) * BS])
        m["y"] = np.ascontiguousarray(Y[c * BS:(c + 1) * BS])
        in_maps.append(m)
    res = run_bass_kernel_spmd(nc, in_maps, core_ids=list(range(NCORES)), trace=trace)
    out = np.concatenate([np.asarray(res.results[i]["out"]) for i in range(NCORES)],
                         axis=0).astype(np.float32)
    return out, res


def kernel(**inputs):
    out, _ = _run(inputs, trace=False)
    return out


# revision 15
# speedup vs baseline: 1.0029x; 1.0020x over previous
"""DA-RNN (dual-stage attention RNN) forward pass on 8 TRN2 NeuronCores.

Data-parallel: batch 2048 sharded 256 per core, weights replicated.

Algebraic structure exploited (validated against the reference in numpy):
  * Both attention blocks add their state-dependent term as a per-sample
    constant across the softmax axis, so softmax cancels it.  The encoder
    input attention (a1, a2) and the decoder temporal attention (beta) are
    input-only precomputes, and the decoder context vector is constant
    across decoder steps.
  * context only enters through dot products (fc_W, fc_final_W): three
    matvec columns [v, fcW1, w_c] against X_encoded give score/q/r per
    (sample, t); softmax-weighted sums of q, r give the decoder LSTM input
    offset (A) and the output contribution.
  * The decoder LSTM input is scalar per sample.  It is fed as a K=3
    augmented matmul with rhs rows [w_y*y_t; A; 1] against lhsT rows
    [Wd0; Wd0; bd + Wd0*fc_b]: the y rows and ones rows are input-only and
    are staged into the 4 row-tiled strips BEFORE the encoder runs; only
    the single A row (shared by all steps) is written after the encoder,
    so the encoder->decoder transition has a ~2us serial path instead of a
    serialized 36-DMA storm.
  * softmax exp is computed via the sigmoid identity e^x = s/(1-s)
    (scores are O(0.3)), so the Scalar engine never swaps activation
    tables (Exp <-> Sigmoid/Tanh swaps cost 1.3us each on the serial
    path).

Precision plan (validated numerically on the fixed inputs):
  * All hidden (recurrent) weights are fp8e4 (halves weight DMA);
    states stay fp16 and stream as the matmul moving operand (mixed
    fp8-weight x fp16-state matmuls run at full bf16 rate and register as
    activity to the HAM PE clock gate, keeping the array at 2.4 GHz).
  * Input matmuls (K=17/16/3) are packed 4x concurrent via row-tiled
    tile_position strips with replicated weights/rhs.
  * Within a PSUM bank, each accumulation group's chain completes before
    the next group's start=True (a start clears has_written bank-wide).

HAM discipline: the PE clock gate needs sustained activity; dummy/dep-
gated warm matmuls bridge every >3.4us PE-idle window (DMA preamble,
attention precompute, encoder->decoder transition).

On-chip layout: feature-major [dim on partitions (128-chunks), batch on
free].  LSTM states stay in that layout so no transposes in the
recurrences.  PSUM accumulates fp32 throughout.
"""

import sys
import os

sys.path.insert(0, "/opt/trn_rl_repo")
os.environ.setdefault("MYCRO_LOCAL_CACHE", "1")

from contextlib import ExitStack

import numpy as np
import ml_dtypes

import concourse.bass as bass
import concourse.mybir as mybir
import concourse.tile as tile
from concourse import bacc
from concourse.bass_utils import run_bass_kernel_spmd
from concourse.masks import make_identity

F32 = mybir.dt.float32
F16 = mybir.dt.float16
F8 = mybir.dt.float8e4
AF = mybir.ActivationFunctionType
ALU = mybir.AluOpType

NCORES = 8
P = 128
BS = 256          # batch per core
NJ = 2            # 128-partition batch chunks
T = 9             # recurrence steps (T-1 in the reference)
H = 512
IN2 = 15
ME = 16           # encoder gate chunks (4H/128)
KE = 4            # encoder hidden chunks (H/128)
MD = 32           # decoder gate chunks (8H/128)
KD = 8            # decoder hidden chunks (2H/128)

N_WARM_PREFIX = 12
T_PRE = T         # x-tilde transpose steps done before the recurrence


def _np(a):
    return np.asarray(a, dtype=np.float32)


def _f16(a):
    return np.ascontiguousarray(np.asarray(a, dtype=np.float32).astype(np.float16))


def _f8(a):
    return np.ascontiguousarray(
        np.asarray(a, dtype=np.float32).astype(ml_dtypes.float8_e4m3))


def _pack_weights(inp):
    """Host-side weight folding (weight-only transforms; no input math)."""
    Wih1, Whh1 = _np(inp["enc_lstm_Wih"]), _np(inp["enc_lstm_Whh"])
    b1 = _np(inp["enc_lstm_bih"]) + _np(inp["enc_lstm_bhh"])
    Wih2, Whh2 = _np(inp["enc_lstm1_Wih"]), _np(inp["enc_lstm1_Whh"])
    b2 = _np(inp["enc_lstm1_bih"]) + _np(inp["enc_lstm1_bhh"])
    Wd_ih, Wd_hh = _np(inp["dec_lstm_Wih"]), _np(inp["dec_lstm_Whh"])
    bd = _np(inp["dec_lstm_bih"]) + _np(inp["dec_lstm_bhh"])
    attn1_W = _np(inp["dec_attn1_W"])
    attn2_w = _np(inp["dec_attn2_W"])[0]
    fc_W = _np(inp["fc_W"])[0]
    fcf_W = _np(inp["fc_final_W"])[0]

    W1x = attn1_W[:, 4 * H:]                        # (512, 1024)
    v = W1x.T @ attn2_w                             # (1024,)
    fcW1 = fc_W[:2 * H]
    w_c = fcf_W[2 * H:]
    w_d = fcf_W[:2 * H]

    # input+bias weights, replicated at 4 row strips for row-tiled quads
    wia1 = np.concatenate([Wih1.T, b1[None, :]], axis=0)            # (17, 2048)
    wia2 = np.concatenate([Wih2.T, b2[None, :]], axis=0)            # (16, 2048)
    Wd0 = Wd_ih[:, 0]
    wid3 = np.stack([Wd0, Wd0,
                     bd + Wd0 * float(_np(inp["fc_b"])[0])],
                    axis=0)                                         # (3, 4096)

    def _part(w, k):  # (k*P, m) -> (P, k, m) partition-contiguous
        m = w.shape[1]
        return np.ascontiguousarray(
            w.reshape(k, P, m).transpose(1, 0, 2))

    def _strips(w):  # (nk, m) -> (P, m) replicated at partition strips 32g
        nk, m = w.shape
        out = np.zeros((P, m), np.float32)
        for g in range(4):
            out[32 * g:32 * g + nk] = w
        return out

    weights = {
        "wia1": _f16(_strips(wia1)),
        "wia2": _f16(_strips(wia2)),
        "wid3": _f16(_strips(wid3)),
        "whh1": _f8(_part(Whh1.T, KE)),                             # (P, KE, 2048)
        "whh2": _f8(_part(Whh2.T, KE)),                             # (P, KE, 2048)
        "whhd": _f8(_part(Wd_hh.T, KD)),                            # (P, KD, 4096)
        "v3": _f16(_part(np.stack([v, fcW1, w_c], axis=1), KD)),    # (P, KD, 3)
        "wd": _f16(_part(w_d[:, None], KD)),                        # (P, KD, 1)
    }
    scalars = {
        "Wf": [float(x) for x in _np(inp["enc_attn_W"])[0, 2 * H:]],    # 9 floats
        "w_y": float(fc_W[2 * H]),
        "fcf_b": float(_np(inp["fc_final_b"])[0]),
    }
    return weights, scalars


def _build(scal, upto="full"):
    nc = bacc.Bacc()

    xd = nc.declare_dram_parameter("x", [BS, T, IN2], F32, isOutput=False)
    yd = nc.declare_dram_parameter("y", [BS, T], F32, isOutput=False)
    wia1d = nc.declare_dram_parameter("wia1", [P, 4 * H], F16, isOutput=False)
    wia2d = nc.declare_dram_parameter("wia2", [P, 4 * H], F16, isOutput=False)
    wid3d = nc.declare_dram_parameter("wid3", [P, 8 * H], F16, isOutput=False)
    whh1d = nc.declare_dram_parameter("whh1", [P, KE, 4 * H], F8, isOutput=False)
    whh2d = nc.declare_dram_parameter("whh2", [P, KE, 4 * H], F8, isOutput=False)
    whhdd = nc.declare_dram_parameter("whhd", [P, KD, 8 * H], F8, isOutput=False)
    v3d = nc.declare_dram_parameter("v3", [P, KD, 3], F16, isOutput=False)
    wdd = nc.declare_dram_parameter("wd", [P, KD, 1], F16, isOutput=False)
    outd = nc.declare_dram_parameter("out", [BS, 1], F32, isOutput=True)
    dbgd = (nc.declare_dram_parameter("dbg", [BS, T, 3], F32, isOutput=True)
            if upto == "enc" else None)

    Wf = scal["Wf"]

    with ExitStack() as ctx:
        tc = ctx.enter_context(tile.TileContext(nc))
        # persistent pools
        pw = ctx.enter_context(tc.tile_pool(name="pw", bufs=1))
        psm = ctx.enter_context(tc.tile_pool(name="psm", bufs=4))     # small f32 scratch
        pu = ctx.enter_context(tc.tile_pool(name="pu", bufs=4))       # cell temp
        pya = ctx.enter_context(tc.tile_pool(name="pya", bufs=1))
        psum_g = ctx.enter_context(tc.tile_pool(name="psum_g", bufs=5, space="PSUM"))
        psum_t = ctx.enter_context(tc.tile_pool(name="psum_t", bufs=1, space="PSUM"))
        psum_q = ctx.enter_context(tc.tile_pool(name="psum_q", bufs=2, space="PSUM"))

        # ---------------- input DMAs (critical path first) ----------------
        yb = pw.tile([P, NJ, T], F32)
        nc.sync.dma_start(out=yb, in_=yd.rearrange("(j p) t -> p j t", p=P))
        xb0 = pw.tile([P, NJ, T, IN2], F32)
        xd_r = xd.rearrange("(j p) t f -> p j t f", p=P)
        for j in range(NJ):
            nc.sync.dma_start(out=xb0[:, j, :, :], in_=xd_r[:, j, :, :])
        # identity/filler tiles BEFORE the gpsimd DMA block: make_identity
        # runs on gpsimd, and the PE warm-up fills need it immediately
        ident = pw.tile([P, P], F16)
        make_identity(nc, ident)
        wbig = pw.tile([P, 480], F16)
        nc.vector.memset(wbig, 1.0)

        # encoder hidden weights on the gpsimd queue; nothing else is
        # emitted on gpsimd before the encoder so the hidden matmuls only
        # wait for these two transfers (consumers wait on coarse per-queue
        # DMA completion counters, so emission order = dependency scope)
        whh1 = pw.tile([P, KE, 4 * H], F8)
        nc.gpsimd.dma_start(out=whh1, in_=whh1d[:, :, :])
        whh2 = pw.tile([P, KE, 4 * H], F8)
        nc.gpsimd.dma_start(out=whh2, in_=whh2d[:, :, :])

        # PE warm-up helpers.  The HAM clock gate needs sustained activity;
        # _warm is un-gated (runs immediately at its queue position),
        # _warm_on is gated on a data tile so it fires mid-pipeline.
        def _warm(n):
            for _ in range(n):
                wps = psum_q.tile([P, P], F32, name="wps", tag="psq")
                nc.tensor.matmul(wps, ident, ident, start=True, stop=True)

        def _wfill(n):
            # dense filler: 480-col streams keep the HAM activity monitor
            # fed (un-throttle needs ~27us of SUSTAINED busy on this part,
            # so every idle window must be prevented, not repaired)
            for _ in range(n):
                wps = psum_q.tile([P, 480], F32, name="wf", tag="psq")
                nc.tensor.matmul(wps, ident, wbig, start=True, stop=True)

        wseq = [0]

        def _warm_on(lhs, rhs, n=1):
            m = lhs.free_size()
            nfree = rhs.free_size()
            for _ in range(n):
                wseq[0] += 1
                wps = psum_q.tile([min(m, P), min(nfree, 480)], F32,
                                  name=f"wo{wseq[0]}", tag="psq")
                nc.tensor.matmul(wps, lhs, rhs, start=True, stop=True)

        _wfill(6)

        # persistent state / small tiles
        sq2 = pw.tile([P, NJ, T, 2], F32)     # q, r matvec results
        es = pw.tile([P, NJ, T], F32)         # sigmoid(score) per step
        A_t = pw.tile([P, NJ], F32)
        A16 = pw.tile([P, NJ], F16)
        ctxw = pw.tile([P, NJ], F32)
        ytldT3 = pw.tile([3 * T, BS], F16)    # [w_y*y_t; 0; 1] transposed
        arowT = pw.tile([1, BS], F16)         # A row, post-encoder
        cd = pw.tile([P, KD, BS], F32)
        osb = pw.tile([P, NJ, 1], F32)
        ytA = [pya.tile([P, BS], F16, name=f"ytA{t}", tag=f"ytA{t}")
               for t in range(T)]

        # ---------------- decoder input rows (pre-encoder part) -----------
        # ytld3 free layout f = 3t + r with rows [w_y*y_t; A=0; 1]; the
        # transpose then yields ytldT3 rows 3t..3t+2 = that triple.
        ytld3 = pw.tile([P, NJ, 3 * T], F16)
        nc.vector.memset(ytld3, 1.0)
        nc.vector.memset(ytld3[:, :, 1::3], 0.0)
        nc.vector.tensor_scalar_mul(out=ytld3[:, :, 0::3], in0=yb,
                                    scalar1=scal["w_y"])
        for j in range(NJ):
            tpy = psum_t.tile([3 * T, P], F16, name="tpy", tag="pst")
            nc.tensor.transpose(tpy, ytld3[:, j, :], ident)
            nc.scalar.copy(out=ytldT3[:, j * P:(j + 1) * P], in_=tpy)

        _warm_on(yb[:, 0, :], yb, 1)
        _wfill(3)

        with tc.tile_pool(name="penc", bufs=1) as penc:
            c1 = penc.tile([P, KE, BS], F32)
            c2 = penc.tile([P, KE, BS], F32)
            xtA1 = [penc.tile([P, BS], F16, name=f"xa1_{t}", tag=f"xa1_{t}")
                    for t in range(T)]
            xtA2 = [penc.tile([P, BS], F16, name=f"xa2_{t}", tag=f"xa2_{t}")
                    for t in range(T)]
            xt1 = penc.tile([P, NJ, T, 17], F16)
            xt2 = penc.tile([P, NJ, T, 16], F16)

            def _xtile_transpose(t, evac_engine):
                # 4x col-tiled concurrent transposes into partition strips
                # 0/32/64/96 of one PSUM tile, evacuated with a single
                # full-width copy: row-strip replicas for the 4x row-tiled
                # input matmuls with no SBUF->SBUF DMA storm.
                for j in range(NJ):
                    tp1 = psum_g.tile([P, P], F16, name="tp1", tag="psg")
                    for g in range(4):
                        nc.tensor.transpose(tp1[32 * g:32 * g + 17, :],
                                            xt1[:, j, t, :], ident,
                                            tile_position=(0, 32 * g))
                    if evac_engine == "split":
                        nc.scalar.copy(out=xtA1[t][:, j * P:(j + 1) * P],
                                       in_=tp1)
                    else:
                        nc.vector.tensor_copy(
                            out=xtA1[t][:, j * P:(j + 1) * P], in_=tp1)
                    tp2 = psum_g.tile([P, P], F16, name="tp2", tag="psg")
                    for g in range(4):
                        nc.tensor.transpose(tp2[32 * g:32 * g + 16, :],
                                            xt2[:, j, t, :], ident,
                                            tile_position=(0, 32 * g))
                    nc.vector.tensor_copy(out=xtA2[t][:, j * P:(j + 1) * P],
                                          in_=tp2)

            with tc.tile_pool(name="ptmp", bufs=1) as ptmp:
                # ---------------- encoder attention precompute ------------
                xyb = ptmp.tile([P, NJ, T, 16], F32)
                nc.vector.tensor_copy(out=xyb[:, :, :, 0:IN2], in_=xb0)
                nc.vector.tensor_copy(out=xyb[:, :, :, IN2], in_=yb[:, :, :])
                _warm_on(xb0[:, 0, 0, :], xb0[:, 1, :, :], 1)
                _wfill(3)
                mmb = ptmp.tile([P, NJ, T, IN2], F32)
                nc.vector.tensor_mul(
                    out=mmb,
                    in0=xyb[:, :, :, 0:IN2],
                    in1=yb.unsqueeze(3).to_broadcast([P, NJ, T, IN2]),
                )
                # Wf-weighted sums over t in 3 independent sub-chains per
                # base so the DVE pipeline isn't one long dependency chain.
                bases = []
                for bi, src_ in ((0, xyb), (1, mmb)):
                    Fb = 16 if bi == 0 else IN2
                    parts = []
                    for c in range(3):
                        pb = ptmp.tile([P, NJ, Fb], F32, name=f"b{bi}p{c}")
                        nc.vector.tensor_scalar_mul(out=pb,
                                                    in0=src_[:, :, 3 * c, :],
                                                    scalar1=Wf[3 * c])
                        for t in (3 * c + 1, 3 * c + 2):
                            nc.vector.scalar_tensor_tensor(
                                out=pb, in0=src_[:, :, t, :], scalar=Wf[t],
                                in1=pb, op0=ALU.mult, op1=ALU.add)
                        parts.append(pb)
                    nc.vector.tensor_add(out=parts[0], in0=parts[0], in1=parts[1])
                    nc.vector.tensor_add(out=parts[0], in0=parts[0], in1=parts[2])
                    bases.append(parts[0])
                    if bi == 0:
                        _warm_on(xyb[:, 0, 0, :], xyb[:, 1, :, :], 1)
                        _wfill(3)
                base1, base2 = bases
                _warm_on(mmb[:, 0, 0, :], mmb[:, 1, :, :], 1)
                _wfill(3)

                # softmax via sigmoid identity: e^x = s/(1-s); no Exp table
                # is ever loaded so the Scalar engine never swaps tables.
                a1 = ptmp.tile([P, NJ, 16], F32)
                a2 = ptmp.tile([P, NJ, IN2], F32)
                for bi, (base, a) in enumerate(((base1, a1), (base2, a2))):
                    Fb = 16 if bi == 0 else IN2
                    for j in range(NJ):
                        sg = psm.tile([P, Fb], F32, name="sg",
                                      tag=f"sg{bi}{j}")
                        nc.scalar.activation(out=sg, in_=base[:, j, :],
                                             func=AF.Sigmoid)
                        om = psm.tile([P, Fb], F32, name="om",
                                      tag=f"om{bi}{j}")
                        nc.vector.tensor_scalar_mul(out=om, in0=sg,
                                                    scalar1=-1.0)
                        nc.vector.tensor_scalar_add(out=om, in0=om,
                                                    scalar1=1.0)
                        nc.vector.reciprocal(out=om, in_=om)
                        nc.vector.tensor_mul(out=a[:, j, :], in0=sg, in1=om)
                        ssum = psm.tile([P, 1], F32, name="ssum",
                                        tag=f"ssum{bi}{j}")
                        nc.vector.reduce_sum(out=ssum, in_=a[:, j, :],
                                             axis=mybir.AxisListType.X)
                        inv = psm.tile([P, 1], F32, name="inv", tag=f"inv{bi}{j}")
                        nc.vector.reciprocal(out=inv, in_=ssum)
                        nc.vector.tensor_scalar_mul(out=a[:, j, :], in0=a[:, j, :],
                                                    scalar1=inv)

                _warm_on(a1[:, 0, :], a1[:, 1, :], 1)
                _wfill(3)
                # last column = 1.0 so the transpose yields the ones row that
                # carries the bias through the K-augmented matmul
                nc.vector.memset(xt1[:, :, :, 16:17], 1.0)
                nc.vector.tensor_mul(
                    out=xt1[:, :, :, 0:16], in0=xyb,
                    in1=a1.unsqueeze(2).to_broadcast([P, NJ, T, 16]))
                nc.vector.memset(xt2[:, :, :, IN2:16], 1.0)
                nc.vector.tensor_mul(
                    out=xt2[:, :, :, 0:IN2], in0=mmb,
                    in1=a2.unsqueeze(2).to_broadcast([P, NJ, T, IN2]))

                _warm_on(ident, xt1[:, 0, :, :], 1)
                _wfill(3)
                if upto == "pre":
                    nc.vector.tensor_copy(out=osb, in_=xt1[:, :, 0, 0:1])

            wiaS1 = pw.tile([P, 4 * H], F16)
            nc.sync.dma_start(out=wiaS1, in_=wia1d[:, :])
            wiaS2 = pw.tile([P, 4 * H], F16)
            nc.sync.dma_start(out=wiaS2, in_=wia2d[:, :])
            v3t = pw.tile([P, KD, 3], F16)
            nc.sync.dma_start(out=v3t, in_=v3d[:, :, :])
            wdt = pw.tile([P, KD, 1], F16)
            nc.sync.dma_start(out=wdt, in_=wdd[:, :, :])

            # first T_PRE steps' transposes before the recurrence; the rest
            # are interleaved into the recurrence loop (step t emits t+T_PRE)
            for t in range(T_PRE):
                _xtile_transpose(t, "split")
                if t < 4:
                    _wfill(2)

            # staged decoder-input rows: [w_y*y_t; 0; 1] into the 4 strips
            # (overlaps the encoder; the A row lands post-encoder)
            for t in range(T):
                for g in range(4):
                    nc.sync.dma_start(out=ytA[t][32 * g:32 * g + 3, :],
                                      in_=ytldT3[3 * t:3 * t + 3, :])

            # ---------------- encoder recurrence + score matvecs ----------
            # States in hidden-chunk pairs [P, 2, BS] fp16: rhs layout for
            # the hidden matmuls and lhsT for the score matvecs.
            with tc.tile_pool(name="px16", bufs=3) as px16, \
                 tc.tile_pool(name="pg", bufs=5) as pg:
                prev16 = None
                for t in range(T if upto != "pre" else 0):
                    if t + T_PRE < T:
                        _xtile_transpose(t + T_PRE, "vector")
                    # HAM: t=0 is input-only (choppy waves) and t=0/t=1
                    # cell math leaves the PE idle; warm bursts keep the
                    # clock gate at 8/8 so t=1/t=2 stream at 2.4 GHz.
                    if t == 1:
                        _wfill(14)
                    elif t == 2:
                        _wfill(6)
                    xe16 = [px16.tile([P, 2, BS], F16, name=f"xe16_{i}",
                                      tag=f"xe16_{i}") for i in range(4)]
                    for br, (wiaS, nk, whhX, cbr) in enumerate((
                            (wiaS1, 17, whh1, c1),
                            (wiaS2, 16, whh2, c2))):
                        xtA = xtA1[t] if br == 0 else xtA2[t]
                        for kp in range(KE // 2):
                            pss = [psum_g.tile([P, 2, BS], F32, name=f"ps{g}",
                                               tag="psg") for g in range(4)]
                            # Per half: 4x concurrent row-tiled input
                            # matmuls, then the fp8-weight x fp16-state
                            # accumulates.  Each half's chain completes
                            # before the next half's start=True (a start
                            # clears has_written for the whole bank).
                            for half in range(2):
                                for g in range(4):
                                    m = g * KE + 2 * kp + half
                                    nc.tensor.matmul(
                                        pss[g][:, half, :],
                                        wiaS[32 * g:32 * g + nk,
                                             m * P:(m + 1) * P],
                                        xtA[32 * g:32 * g + nk, :],
                                        start=True, stop=(t == 0),
                                        tile_position=(32 * g, 0))
                                if t > 0:
                                    for g in range(4):
                                        m = g * KE + 2 * kp + half
                                        for k in range(KE):
                                            nc.tensor.matmul(
                                                pss[g][:, half, :],
                                                whhX[:, k, m * P:(m + 1) * P],
                                                prev16[2 * br + k // 2][:, k % 2, :],
                                                start=False, stop=(k == KE - 1))
                            if t == 0:
                                _wfill(2)
                            gt = pg.tile([P, 4, 2, BS], F16, name="gt", tag="ge")
                            for g in range(4):
                                fn = AF.Tanh if g == 2 else AF.Sigmoid
                                nc.scalar.activation(out=gt[:, g, :, :],
                                                     in_=pss[g], func=fn)
                            cs = cbr[:, 2 * kp:2 * kp + 2, :]
                            if t == 0:
                                nc.vector.tensor_mul(out=cs, in0=gt[:, 0, :, :],
                                                     in1=gt[:, 2, :, :])
                            else:
                                u = pu.tile([P, 2, BS], F32, name="u", tag="u")
                                nc.vector.tensor_mul(out=u, in0=gt[:, 0, :, :],
                                                     in1=gt[:, 2, :, :])
                                nc.vector.tensor_mul(out=cs, in0=gt[:, 1, :, :],
                                                     in1=cs)
                                nc.vector.tensor_add(out=cs, in0=cs, in1=u)
                            nc.scalar.activation(out=gt[:, 2, :, :], in_=cs,
                                                 func=AF.Tanh)
                            xi = 2 * br + kp
                            nc.vector.tensor_mul(out=xe16[xi],
                                                 in0=gt[:, 3, :, :],
                                                 in1=gt[:, 2, :, :])
                    # score/q/r matvecs against the 3 packed columns; the
                    # score column goes through sigmoid NOW so the softmax
                    # after the encoder is pure Vector math.
                    for j in range(NJ):
                        psq = psum_q.tile([P, 3], F32, name="psq", tag="psq")
                        for k in range(KD):
                            nc.tensor.matmul(psq,
                                             xe16[k // 2][:, k % 2,
                                                          j * P:(j + 1) * P],
                                             v3t[:, k, :],
                                             start=(k == 0), stop=(k == KD - 1))
                        nc.scalar.activation(out=es[:, j, t:t + 1],
                                             in_=psq[:, 0:1], func=AF.Sigmoid)
                        nc.vector.tensor_copy(out=sq2[:, j, t, :],
                                              in_=psq[:, 1:3])
                    prev16 = xe16

            # decoder weights: emitted here (consumers are decoder-only) but
            # executed by the gpsimd queue during the encoder
            wid3R = pw.tile([P, 8 * H], F16)
            nc.gpsimd.dma_start(out=wid3R, in_=wid3d[:, :])
            whhd = pw.tile([P, KD, 8 * H], F8)
            nc.gpsimd.dma_start(out=whhd, in_=whhdd[:, :, :])

            # ------------- decoder attention (post-encoder, ~2us) ---------
            # keep the PE busy right at the start of the gap
            if upto in ("beta", "dec", "full"):
                _wfill(8)
                _warm_on(ident[0:3 * T, :], ytldT3, 1)
            if upto == "enc":
                nc.vector.tensor_copy(out=osb, in_=es[:, :, 0:1])
                dbgb = pw.tile([P, NJ, T, 3], F32, name="dbgb")
                nc.vector.tensor_copy(out=dbgb[:, :, :, 0], in_=es)
                nc.vector.tensor_copy(out=dbgb[:, :, :, 1:3], in_=sq2)
                nc.sync.dma_start(out=dbgd.rearrange("(j p) t c -> p j t c", p=P),
                                  in_=dbgb)
            for j in range(NJ if upto in ("beta", "dec", "full") else 0):
                # e_t = s/(1-s); beta-weighted sums of q (A) and r (ctxw)
                e = psm.tile([P, T], F32, name="e", tag=f"e{j}")
                om = psm.tile([P, T], F32, name="omb", tag=f"omb{j}")
                nc.vector.tensor_scalar_mul(out=om, in0=es[:, j, :],
                                            scalar1=-1.0)
                nc.vector.tensor_scalar_add(out=om, in0=om, scalar1=1.0)
                nc.vector.reciprocal(out=om, in_=om)
                _warm_on(om, om, 1)
                _wfill(2)
                nc.vector.tensor_mul(out=e, in0=es[:, j, :], in1=om)
                ssum = psm.tile([P, 1], F32, name="ssum", tag=f"bsum{j}")
                nc.vector.reduce_sum(out=ssum, in_=e, axis=mybir.AxisListType.X)
                _warm_on(e, e, 1)
                _wfill(2)
                inv = psm.tile([P, 1], F32, name="inv", tag=f"binv{j}")
                nc.vector.reciprocal(out=inv, in_=ssum)
                tmp9 = psm.tile([P, T], F32, name="tmp9", tag=f"tmp9{j}")
                eq = psm.tile([P, 1], F32, name="eq", tag=f"eq{j}")
                nc.vector.tensor_mul(out=tmp9, in0=e, in1=sq2[:, j, :, 0])
                nc.vector.reduce_sum(out=eq, in_=tmp9, axis=mybir.AxisListType.X)
                nc.vector.tensor_scalar_mul(out=A_t[:, j:j + 1], in0=eq,
                                            scalar1=inv)
                tmp9b = psm.tile([P, T], F32, name="tmp9b", tag=f"tmp9b{j}")
                er = psm.tile([P, 1], F32, name="er", tag=f"er{j}")
                nc.vector.tensor_mul(out=tmp9b, in0=e, in1=sq2[:, j, :, 1])
                nc.vector.reduce_sum(out=er, in_=tmp9b, axis=mybir.AxisListType.X)
                nc.vector.tensor_scalar_mul(out=er, in0=er, scalar1=inv)
                # ctxw = er + fcf_b  (bias folded so the tail is one op)
                nc.vector.tensor_scalar_add(out=ctxw[:, j:j + 1], in0=er,
                                            scalar1=scal["fcf_b"])

            # A row -> the 4 strips of every step tile (36 x 512B DMAs,
            # t-ascending and split across two idle queues, so decoder t=0
            # starts ~1us after A is known)
            if upto in ("dec", "full"):
                nc.vector.tensor_copy(out=A16, in_=A_t)
                arps = psum_q.tile([1, BS], F16, name="arps", tag="psq")
                for j in range(NJ):
                    nc.tensor.transpose(arps[:, j * P:(j + 1) * P],
                                        A16[:, j:j + 1], ident)
                nc.scalar.copy(out=arowT, in_=arps)
                _warm_on(ident[0:1, :], arowT, 1)
                _wfill(2)
                for t in range(T):
                    for g in range(4):
                        eng = nc.sync if (t * 4 + g) % 2 == 0 else nc.gpsimd
                        eng.dma_start(
                            out=ytA[t][32 * g + 1:32 * g + 2, :],
                            in_=arowT)

        if upto == "beta":
            nc.vector.tensor_copy(out=osb, in_=A_t.unsqueeze(2))

        # ---------------- decoder recurrence ----------------
        ndec = T if upto in ("dec", "full") else 0
        dT16p = None
        pdt16 = ctx.enter_context(tc.tile_pool(name="pdt16", bufs=2))
        with tc.tile_pool(name="pgd", bufs=5) as pgd:
            for t in range(ndec):
                if t == 1:
                    _wfill(14)
                elif t == 2:
                    _wfill(6)
                dprev16 = dT16p
                dT16p = [pdt16.tile([P, 2, BS], F16, name=f"dT16_{i}",
                                    tag=f"dT16_{i}") for i in range(KD // 2)]
                for kp in range(KD // 2):
                    pss = [psum_g.tile([P, 2, BS], F32, name=f"psd{g}",
                                       tag="psg") for g in range(4)]
                    for half in range(2):
                        for g in range(4):
                            m = g * KD + 2 * kp + half
                            nc.tensor.matmul(
                                pss[g][:, half, :],
                                wid3R[32 * g:32 * g + 3, m * P:(m + 1) * P],
                                ytA[t][32 * g:32 * g + 3, :],
                                start=True, stop=(t == 0),
                                tile_position=(32 * g, 0))
                        if t > 0:
                            for g in range(4):
                                m = g * KD + 2 * kp + half
                                for k in range(KD):
                                    nc.tensor.matmul(
                                        pss[g][:, half, :],
                                        whhd[:, k, m * P:(m + 1) * P],
                                        dprev16[k // 2][:, k % 2, :],
                                        start=False, stop=(k == KD - 1))
                    if t == 0:
                        _wfill(2)
                    gt = pgd.tile([P, 4, 2, BS], F16, name="gtd", tag="gd")
                    for g in range(4):
                        fn = AF.Tanh if g == 2 else AF.Sigmoid
                        nc.scalar.activation(out=gt[:, g, :, :], in_=pss[g],
                                             func=fn)
                    cs = cd[:, 2 * kp:2 * kp + 2, :]
                    if t == 0:
                        nc.vector.tensor_mul(out=cs, in0=gt[:, 0, :, :],
                                             in1=gt[:, 2, :, :])
                    else:
                        u = pu.tile([P, 2, BS], F32, name="ud", tag="u")
                        nc.vector.tensor_mul(out=u, in0=gt[:, 0, :, :],
                                             in1=gt[:, 2, :, :])
                        nc.vector.tensor_mul(out=cs, in0=gt[:, 1, :, :], in1=cs)
                        nc.vector.tensor_add(out=cs, in0=cs, in1=u)
                    nc.scalar.activation(out=gt[:, 2, :, :], in_=cs, func=AF.Tanh)
                    nc.vector.tensor_mul(out=dT16p[kp], in0=gt[:, 3, :, :],
                                         in1=gt[:, 2, :, :])

        # ---------------- output ----------------
        if upto == "dec":
            nc.vector.tensor_copy(out=osb, in_=cd[:, 0:NJ, 0:1])
        if upto == "full":
            for j in range(NJ):
                psf = psum_q.tile([P, 1], F32, name="psf", tag="psq")
                for k in range(KD):
                    nc.tensor.matmul(psf,
                                     dT16p[k // 2][:, k % 2, j * P:(j + 1) * P],
                                     wdt[:, k, :],
                                     start=(k == 0), stop=(k == KD - 1))
                nc.vector.scalar_tensor_tensor(
                    out=osb[:, j, :], in0=psf, scalar=1.0,
                    in1=ctxw[:, j:j + 1], op0=ALU.mult, op1=ALU.add)
        nc.sync.dma_start(out=outd.rearrange("(j p) c -> p j c", p=P), in_=osb)

    nc.compile()
    return nc


def _run(inputs, trace=False, upto="full"):
    weights, scal = _pack_weights(inputs)
    nc = _build(scal, upto=upto)
    X = np.ascontiguousarray(_np(inputs["X"]))
    Y = np.ascontiguousarray(_np(inputs["y_prev"]))
    in_maps = []
    for c in range(NCORES):
        m = dict(weights)
        m["x"] = np.ascontiguousarray(X[c * BS:(c + 1) * BS])
        m["y"] = np.ascontiguousarray(Y[c * BS:(c + 1) * BS])
        in_maps.append(m)
    res = run_bass_kernel_spmd(nc, in_maps, core_ids=list(range(NCORES)), trace=trace)
    out = np.concatenate([np.asarray(res.results[i]["out"]) for i in range(NCORES)],
                         axis=0).astype(np.float32)
    return out, res


def kernel(**inputs):
    out, _ = _run(inputs, trace=False)
    return out


# revision 16
# speedup vs baseline: 1.0073x; 1.0044x over previous
"""DA-RNN (dual-stage attention RNN) forward pass on 8 TRN2 NeuronCores.

Data-parallel: batch 2048 sharded 256 per core, weights replicated.

Algebraic structure exploited (validated against the reference in numpy):
  * Both attention blocks add their state-dependent term as a per-sample
    constant across the softmax axis, so softmax cancels it.  The encoder
    input attention (a1, a2) and the decoder temporal attention (beta) are
    input-only precomputes, and the decoder context vector is constant
    across decoder steps.
  * context only enters through dot products (fc_W, fc_final_W): three
    matvec columns [v, fcW1, w_c] against X_encoded give score/q/r per
    (sample, t); softmax-weighted sums of q, r give the decoder LSTM input
    offset (A) and the output contribution.
  * The decoder LSTM input is scalar per sample.  It is fed as a K=3
    augmented matmul with rhs rows [w_y*y_t; A; 1] against lhsT rows
    [Wd0; Wd0; bd + Wd0*fc_b]: the y rows and ones rows are input-only and
    are staged into the 4 row-tiled strips BEFORE the encoder runs; only
    the single A row (shared by all steps) is written after the encoder,
    so the encoder->decoder transition has a ~2us serial path instead of a
    serialized 36-DMA storm.
  * softmax exp is computed via the sigmoid identity e^x = s/(1-s)
    (scores are O(0.3)), so the Scalar engine never swaps activation
    tables (Exp <-> Sigmoid/Tanh swaps cost 1.3us each on the serial
    path).

Precision plan (validated numerically on the fixed inputs):
  * All hidden (recurrent) weights are fp8e4 (halves weight DMA);
    states stay fp16 and stream as the matmul moving operand (mixed
    fp8-weight x fp16-state matmuls run at full bf16 rate and register as
    activity to the HAM PE clock gate, keeping the array at 2.4 GHz).
  * Input matmuls (K=17/16/3) are packed 4x concurrent via row-tiled
    tile_position strips with replicated weights/rhs.
  * Within a PSUM bank, each accumulation group's chain completes before
    the next group's start=True (a start clears has_written bank-wide).

HAM discipline: the PE clock gate needs sustained activity; dummy/dep-
gated warm matmuls bridge every >3.4us PE-idle window (DMA preamble,
attention precompute, encoder->decoder transition).

On-chip layout: feature-major [dim on partitions (128-chunks), batch on
free].  LSTM states stay in that layout so no transposes in the
recurrences.  PSUM accumulates fp32 throughout.
"""

import sys
import os

sys.path.insert(0, "/opt/trn_rl_repo")
os.environ.setdefault("MYCRO_LOCAL_CACHE", "1")

from contextlib import ExitStack

import numpy as np
import ml_dtypes

import concourse.bass as bass
import concourse.mybir as mybir
import concourse.tile as tile
from concourse import bacc
from concourse.bass_utils import run_bass_kernel_spmd
from concourse.masks import make_identity

F32 = mybir.dt.float32
F16 = mybir.dt.float16
F8 = mybir.dt.float8e4
AF = mybir.ActivationFunctionType
ALU = mybir.AluOpType

NCORES = 8
P = 128
BS = 256          # batch per core
NJ = 2            # 128-partition batch chunks
T = 9             # recurrence steps (T-1 in the reference)
H = 512
IN2 = 15
ME = 16           # encoder gate chunks (4H/128)
KE = 4            # encoder hidden chunks (H/128)
MD = 32           # decoder gate chunks (8H/128)
KD = 8            # decoder hidden chunks (2H/128)

N_WARM_PREFIX = 12
T_PRE = T         # x-tilde transpose steps done before the recurrence


def _np(a):
    return np.asarray(a, dtype=np.float32)


def _f16(a):
    return np.ascontiguousarray(np.asarray(a, dtype=np.float32).astype(np.float16))


def _f8(a):
    return np.ascontiguousarray(
        np.asarray(a, dtype=np.float32).astype(ml_dtypes.float8_e4m3))


def _pack_weights(inp):
    """Host-side weight folding (weight-only transforms; no input math)."""
    Wih1, Whh1 = _np(inp["enc_lstm_Wih"]), _np(inp["enc_lstm_Whh"])
    b1 = _np(inp["enc_lstm_bih"]) + _np(inp["enc_lstm_bhh"])
    Wih2, Whh2 = _np(inp["enc_lstm1_Wih"]), _np(inp["enc_lstm1_Whh"])
    b2 = _np(inp["enc_lstm1_bih"]) + _np(inp["enc_lstm1_bhh"])
    Wd_ih, Wd_hh = _np(inp["dec_lstm_Wih"]), _np(inp["dec_lstm_Whh"])
    bd = _np(inp["dec_lstm_bih"]) + _np(inp["dec_lstm_bhh"])
    attn1_W = _np(inp["dec_attn1_W"])
    attn2_w = _np(inp["dec_attn2_W"])[0]
    fc_W = _np(inp["fc_W"])[0]
    fcf_W = _np(inp["fc_final_W"])[0]

    W1x = attn1_W[:, 4 * H:]                        # (512, 1024)
    v = W1x.T @ attn2_w                             # (1024,)
    fcW1 = fc_W[:2 * H]
    w_c = fcf_W[2 * H:]
    w_d = fcf_W[:2 * H]

    # input+bias weights, replicated at 4 row strips for row-tiled quads
    wia1 = np.concatenate([Wih1.T, b1[None, :]], axis=0)            # (17, 2048)
    wia2 = np.concatenate([Wih2.T, b2[None, :]], axis=0)            # (16, 2048)
    Wd0 = Wd_ih[:, 0]
    wid3 = np.stack([Wd0, Wd0,
                     bd + Wd0 * float(_np(inp["fc_b"])[0])],
                    axis=0)                                         # (3, 4096)

    def _part(w, k):  # (k*P, m) -> (P, k, m) partition-contiguous
        m = w.shape[1]
        return np.ascontiguousarray(
            w.reshape(k, P, m).transpose(1, 0, 2))

    def _strips(w):  # (nk, m) -> (P, m) replicated at partition strips 32g
        nk, m = w.shape
        out = np.zeros((P, m), np.float32)
        for g in range(4):
            out[32 * g:32 * g + nk] = w
        return out

    weights = {
        "wia1": _f16(_strips(wia1)),
        "wia2": _f16(_strips(wia2)),
        "wid3": _f16(_strips(wid3)),
        "whh1": _f8(_part(Whh1.T, KE)),                             # (P, KE, 2048)
        "whh2": _f8(_part(Whh2.T, KE)),                             # (P, KE, 2048)
        "whhd": _f8(_part(Wd_hh.T, KD)),                            # (P, KD, 4096)
        "v3": _f16(_part(np.stack([v, fcW1, w_c], axis=1), KD)),    # (P, KD, 3)
        "wd": _f16(_part(w_d[:, None], KD)),                        # (P, KD, 1)
    }
    scalars = {
        "Wf": [float(x) for x in _np(inp["enc_attn_W"])[0, 2 * H:]],    # 9 floats
        "w_y": float(fc_W[2 * H]),
        "fcf_b": float(_np(inp["fc_final_b"])[0]),
    }
    return weights, scalars


def _build(scal, upto="full"):
    nc = bacc.Bacc()

    xd = nc.declare_dram_parameter("x", [BS, T, IN2], F32, isOutput=False)
    yd = nc.declare_dram_parameter("y", [BS, T], F32, isOutput=False)
    wia1d = nc.declare_dram_parameter("wia1", [P, 4 * H], F16, isOutput=False)
    wia2d = nc.declare_dram_parameter("wia2", [P, 4 * H], F16, isOutput=False)
    wid3d = nc.declare_dram_parameter("wid3", [P, 8 * H], F16, isOutput=False)
    whh1d = nc.declare_dram_parameter("whh1", [P, KE, 4 * H], F8, isOutput=False)
    whh2d = nc.declare_dram_parameter("whh2", [P, KE, 4 * H], F8, isOutput=False)
    whhdd = nc.declare_dram_parameter("whhd", [P, KD, 8 * H], F8, isOutput=False)
    v3d = nc.declare_dram_parameter("v3", [P, KD, 3], F16, isOutput=False)
    wdd = nc.declare_dram_parameter("wd", [P, KD, 1], F16, isOutput=False)
    outd = nc.declare_dram_parameter("out", [BS, 1], F32, isOutput=True)
    dbgd = (nc.declare_dram_parameter("dbg", [BS, T, 3], F32, isOutput=True)
            if upto == "enc" else None)

    Wf = scal["Wf"]

    with ExitStack() as ctx:
        tc = ctx.enter_context(tile.TileContext(nc))
        # persistent pools
        pw = ctx.enter_context(tc.tile_pool(name="pw", bufs=1))
        psm = ctx.enter_context(tc.tile_pool(name="psm", bufs=4))     # small f32 scratch
        pu = ctx.enter_context(tc.tile_pool(name="pu", bufs=4))       # cell temp
        pya = ctx.enter_context(tc.tile_pool(name="pya", bufs=1))
        psum_g = ctx.enter_context(tc.tile_pool(name="psum_g", bufs=5, space="PSUM"))
        psum_t = ctx.enter_context(tc.tile_pool(name="psum_t", bufs=1, space="PSUM"))
        psum_q = ctx.enter_context(tc.tile_pool(name="psum_q", bufs=2, space="PSUM"))

        # ---------------- input DMAs (critical path first) ----------------
        yb = pw.tile([P, NJ, T], F32)
        nc.sync.dma_start(out=yb, in_=yd.rearrange("(j p) t -> p j t", p=P))
        xb0 = pw.tile([P, NJ, T, IN2], F32)
        xd_r = xd.rearrange("(j p) t f -> p j t f", p=P)
        for j in range(NJ):
            nc.sync.dma_start(out=xb0[:, j, :, :], in_=xd_r[:, j, :, :])
        # identity/filler tiles BEFORE the gpsimd DMA block: make_identity
        # runs on gpsimd, and the PE warm-up fills need it immediately
        ident = pw.tile([P, P], F16)
        make_identity(nc, ident)
        wbig = pw.tile([P, 480], F16)
        nc.vector.memset(wbig, 1.0)

        # encoder hidden weights on the gpsimd queue; nothing else is
        # emitted on gpsimd before the encoder so the hidden matmuls only
        # wait for these two transfers (consumers wait on coarse per-queue
        # DMA completion counters, so emission order = dependency scope)
        whh1 = pw.tile([P, KE, 4 * H], F8)
        nc.gpsimd.dma_start(out=whh1, in_=whh1d[:, :, :])
        whh2 = pw.tile([P, KE, 4 * H], F8)
        nc.gpsimd.dma_start(out=whh2, in_=whh2d[:, :, :])

        # PE warm-up helpers.  The HAM clock gate needs sustained activity;
        # _warm is un-gated (runs immediately at its queue position),
        # _warm_on is gated on a data tile so it fires mid-pipeline.
        def _warm(n):
            for _ in range(n):
                wps = psum_q.tile([P, P], F32, name="wps", tag="psq")
                nc.tensor.matmul(wps, ident, ident, start=True, stop=True)

        def _wfill(n):
            # dense filler: 480-col streams keep the HAM activity monitor
            # fed (un-throttle needs ~27us of SUSTAINED busy on this part,
            # so every idle window must be prevented, not repaired)
            for _ in range(n):
                wps = psum_q.tile([P, 480], F32, name="wf", tag="psq")
                nc.tensor.matmul(wps, ident, wbig, start=True, stop=True)

        wseq = [0]

        def _warm_on(lhs, rhs, n=1):
            m = lhs.free_size()
            nfree = rhs.free_size()
            for _ in range(n):
                wseq[0] += 1
                wps = psum_q.tile([min(m, P), min(nfree, 480)], F32,
                                  name=f"wo{wseq[0]}", tag="psq")
                nc.tensor.matmul(wps, lhs, rhs, start=True, stop=True)

        _wfill(6)

        # persistent state / small tiles
        sq2 = pw.tile([P, NJ, T, 2], F32)     # q, r matvec results
        es = pw.tile([P, NJ, T], F32)         # sigmoid(score) per step
        A_t = pw.tile([P, NJ], F32)
        A16 = pw.tile([P, NJ], F16)
        ctxw = pw.tile([P, NJ], F32)
        ytldT3 = pw.tile([3 * T, BS], F16)    # [w_y*y_t; 0; 1] transposed
        arowT = pw.tile([1, BS], F16)         # A row, post-encoder
        cd = pw.tile([P, KD, BS], F32)
        osb = pw.tile([P, NJ, 1], F32)
        ytA = [pya.tile([P, BS], F16, name=f"ytA{t}", tag=f"ytA{t}")
               for t in range(T)]

        # ---------------- decoder input rows (pre-encoder part) -----------
        # ytld3 free layout f = 3t + r with rows [w_y*y_t; A=0; 1]; the
        # transpose then yields ytldT3 rows 3t..3t+2 = that triple.
        ytld3 = pw.tile([P, NJ, 3 * T], F16)
        nc.vector.memset(ytld3, 1.0)
        nc.vector.memset(ytld3[:, :, 1::3], 0.0)
        nc.vector.tensor_scalar_mul(out=ytld3[:, :, 0::3], in0=yb,
                                    scalar1=scal["w_y"])
        for j in range(NJ):
            tpy = psum_t.tile([3 * T, P], F16, name="tpy", tag="pst")
            nc.tensor.transpose(tpy, ytld3[:, j, :], ident)
            nc.scalar.copy(out=ytldT3[:, j * P:(j + 1) * P], in_=tpy)

        _warm_on(yb[:, 0, :], yb, 1)
        _wfill(3)

        with tc.tile_pool(name="penc", bufs=1) as penc:
            c1 = penc.tile([P, KE, BS], F32)
            c2 = penc.tile([P, KE, BS], F32)
            xtA1 = [penc.tile([P, BS], F16, name=f"xa1_{t}", tag=f"xa1_{t}")
                    for t in range(T)]
            xtA2 = [penc.tile([P, BS], F16, name=f"xa2_{t}", tag=f"xa2_{t}")
                    for t in range(T)]
            xt1 = penc.tile([P, NJ, T, 17], F16)
            xt2 = penc.tile([P, NJ, T, 16], F16)

            def _xtile_transpose(t, evac_engine):
                # 4x col-tiled concurrent transposes into partition strips
                # 0/32/64/96 of one PSUM tile, evacuated with a single
                # full-width copy: row-strip replicas for the 4x row-tiled
                # input matmuls with no SBUF->SBUF DMA storm.
                for j in range(NJ):
                    tp1 = psum_g.tile([P, P], F16, name="tp1", tag="psg")
                    for g in range(4):
                        nc.tensor.transpose(tp1[32 * g:32 * g + 17, :],
                                            xt1[:, j, t, :], ident,
                                            tile_position=(0, 32 * g))
                    if evac_engine == "split":
                        nc.scalar.copy(out=xtA1[t][:, j * P:(j + 1) * P],
                                       in_=tp1)
                    else:
                        nc.vector.tensor_copy(
                            out=xtA1[t][:, j * P:(j + 1) * P], in_=tp1)
                    tp2 = psum_g.tile([P, P], F16, name="tp2", tag="psg")
                    for g in range(4):
                        nc.tensor.transpose(tp2[32 * g:32 * g + 16, :],
                                            xt2[:, j, t, :], ident,
                                            tile_position=(0, 32 * g))
                    nc.vector.tensor_copy(out=xtA2[t][:, j * P:(j + 1) * P],
                                          in_=tp2)

            with tc.tile_pool(name="ptmp", bufs=1) as ptmp:
                # ---------------- encoder attention precompute ------------
                xyb = ptmp.tile([P, NJ, T, 16], F32)
                nc.vector.tensor_copy(out=xyb[:, :, :, 0:IN2], in_=xb0)
                nc.vector.tensor_copy(out=xyb[:, :, :, IN2], in_=yb[:, :, :])
                _warm_on(xb0[:, 0, 0, :], xb0[:, 1, :, :], 1)
                _wfill(3)
                mmb = ptmp.tile([P, NJ, T, IN2], F32)
                nc.vector.tensor_mul(
                    out=mmb,
                    in0=xyb[:, :, :, 0:IN2],
                    in1=yb.unsqueeze(3).to_broadcast([P, NJ, T, IN2]),
                )
                # Wf-weighted sums over t in 3 independent sub-chains per
                # base so the DVE pipeline isn't one long dependency chain.
                bases = []
                for bi, src_ in ((0, xyb), (1, mmb)):
                    Fb = 16 if bi == 0 else IN2
                    parts = []
                    for c in range(3):
                        pb = ptmp.tile([P, NJ, Fb], F32, name=f"b{bi}p{c}")
                        nc.vector.tensor_scalar_mul(out=pb,
                                                    in0=src_[:, :, 3 * c, :],
                                                    scalar1=Wf[3 * c])
                        for t in (3 * c + 1, 3 * c + 2):
                            nc.vector.scalar_tensor_tensor(
                                out=pb, in0=src_[:, :, t, :], scalar=Wf[t],
                                in1=pb, op0=ALU.mult, op1=ALU.add)
                        parts.append(pb)
                    nc.vector.tensor_add(out=parts[0], in0=parts[0], in1=parts[1])
                    nc.vector.tensor_add(out=parts[0], in0=parts[0], in1=parts[2])
                    bases.append(parts[0])
                    if bi == 0:
                        _warm_on(xyb[:, 0, 0, :], xyb[:, 1, :, :], 1)
                        _wfill(3)
                base1, base2 = bases
                _warm_on(mmb[:, 0, 0, :], mmb[:, 1, :, :], 1)
                _wfill(3)

                # softmax via sigmoid identity: e^x = s/(1-s); no Exp table
                # is ever loaded so the Scalar engine never swaps tables.
                a1 = ptmp.tile([P, NJ, 16], F32)
                a2 = ptmp.tile([P, NJ, IN2], F32)
                for bi, (base, a) in enumerate(((base1, a1), (base2, a2))):
                    Fb = 16 if bi == 0 else IN2
                    for j in range(NJ):
                        sg = psm.tile([P, Fb], F32, name="sg",
                                      tag=f"sg{bi}{j}")
                        nc.scalar.activation(out=sg, in_=base[:, j, :],
                                             func=AF.Sigmoid)
                        om = psm.tile([P, Fb], F32, name="om",
                                      tag=f"om{bi}{j}")
                        nc.vector.tensor_scalar_mul(out=om, in0=sg,
                                                    scalar1=-1.0)
                        nc.vector.tensor_scalar_add(out=om, in0=om,
                                                    scalar1=1.0)
                        nc.vector.reciprocal(out=om, in_=om)
                        nc.vector.tensor_mul(out=a[:, j, :], in0=sg, in1=om)
                        ssum = psm.tile([P, 1], F32, name="ssum",
                                        tag=f"ssum{bi}{j}")
                        nc.vector.reduce_sum(out=ssum, in_=a[:, j, :],
                                             axis=mybir.AxisListType.X)
                        inv = psm.tile([P, 1], F32, name="inv", tag=f"inv{bi}{j}")
                        nc.vector.reciprocal(out=inv, in_=ssum)
                        nc.vector.tensor_scalar_mul(out=a[:, j, :], in0=a[:, j, :],
                                                    scalar1=inv)

                _warm_on(a1[:, 0, :], a1[:, 1, :], 1)
                _wfill(3)
                # last column = 1.0 so the transpose yields the ones row that
                # carries the bias through the K-augmented matmul
                nc.vector.memset(xt1[:, :, :, 16:17], 1.0)
                nc.vector.tensor_mul(
                    out=xt1[:, :, :, 0:16], in0=xyb,
                    in1=a1.unsqueeze(2).to_broadcast([P, NJ, T, 16]))
                nc.vector.memset(xt2[:, :, :, IN2:16], 1.0)
                nc.vector.tensor_mul(
                    out=xt2[:, :, :, 0:IN2], in0=mmb,
                    in1=a2.unsqueeze(2).to_broadcast([P, NJ, T, IN2]))

                _warm_on(ident, xt1[:, 0, :, :], 1)
                _wfill(3)
                if upto == "pre":
                    nc.vector.tensor_copy(out=osb, in_=xt1[:, :, 0, 0:1])

            wiaS1 = pw.tile([P, 4 * H], F16)
            nc.sync.dma_start(out=wiaS1, in_=wia1d[:, :])
            wiaS2 = pw.tile([P, 4 * H], F16)
            nc.sync.dma_start(out=wiaS2, in_=wia2d[:, :])
            v3t = pw.tile([P, KD, 3], F16)
            nc.sync.dma_start(out=v3t, in_=v3d[:, :, :])
            wdt = pw.tile([P, KD, 1], F16)
            nc.sync.dma_start(out=wdt, in_=wdd[:, :, :])

            # first T_PRE steps' transposes before the recurrence; the rest
            # are interleaved into the recurrence loop (step t emits t+T_PRE)
            for t in range(T_PRE):
                _xtile_transpose(t, "split")
                if t < 4:
                    _wfill(2)

            # staged decoder-input rows: [w_y*y_t; 0; 1] into the 4 strips
            # (overlaps the encoder; the A row lands post-encoder)
            for t in range(T):
                for g in range(4):
                    nc.sync.dma_start(out=ytA[t][32 * g:32 * g + 3, :],
                                      in_=ytldT3[3 * t:3 * t + 3, :])

            # ---------------- encoder recurrence + score matvecs ----------
            # States in hidden-chunk pairs [P, 2, BS] fp16: rhs layout for
            # the hidden matmuls and lhsT for the score matvecs.
            with tc.tile_pool(name="px16", bufs=3) as px16, \
                 tc.tile_pool(name="pg", bufs=5) as pg:
                prev16 = None
                for t in range(T if upto != "pre" else 0):
                    if t + T_PRE < T:
                        _xtile_transpose(t + T_PRE, "vector")
                    xe16 = [px16.tile([P, 2, BS], F16, name=f"xe16_{i}",
                                      tag=f"xe16_{i}") for i in range(4)]
                    for br, (wiaS, nk, whhX, cbr) in enumerate((
                            (wiaS1, 17, whh1, c1),
                            (wiaS2, 16, whh2, c2))):
                        xtA = xtA1[t] if br == 0 else xtA2[t]
                        for kp in range(KE // 2):
                            pss = [psum_g.tile([P, 2, BS], F32, name=f"ps{g}",
                                               tag="psg") for g in range(4)]
                            # Per half: 4x concurrent row-tiled input
                            # matmuls, then the fp8-weight x fp16-state
                            # accumulates.  Each half's chain completes
                            # before the next half's start=True (a start
                            # clears has_written for the whole bank).
                            for half in range(2):
                                for g in range(4):
                                    m = g * KE + 2 * kp + half
                                    nc.tensor.matmul(
                                        pss[g][:, half, :],
                                        wiaS[32 * g:32 * g + nk,
                                             m * P:(m + 1) * P],
                                        xtA[32 * g:32 * g + nk, :],
                                        start=True, stop=(t == 0),
                                        tile_position=(32 * g, 0))
                                if t > 0:
                                    for g in range(4):
                                        m = g * KE + 2 * kp + half
                                        for k in range(KE):
                                            nc.tensor.matmul(
                                                pss[g][:, half, :],
                                                whhX[:, k, m * P:(m + 1) * P],
                                                prev16[2 * br + k // 2][:, k % 2, :],
                                                start=False, stop=(k == KE - 1))
                            gt = pg.tile([P, 4, 2, BS], F16, name="gt", tag="ge")
                            for g in range(4):
                                fn = AF.Tanh if g == 2 else AF.Sigmoid
                                nc.scalar.activation(out=gt[:, g, :, :],
                                                     in_=pss[g], func=fn)
                            cs = cbr[:, 2 * kp:2 * kp + 2, :]
                            if t == 0:
                                nc.vector.tensor_mul(out=cs, in0=gt[:, 0, :, :],
                                                     in1=gt[:, 2, :, :])
                            else:
                                u = pu.tile([P, 2, BS], F32, name="u", tag="u")
                                nc.vector.tensor_mul(out=u, in0=gt[:, 0, :, :],
                                                     in1=gt[:, 2, :, :])
                                nc.vector.tensor_mul(out=cs, in0=gt[:, 1, :, :],
                                                     in1=cs)
                                nc.vector.tensor_add(out=cs, in0=cs, in1=u)
                            nc.scalar.activation(out=gt[:, 2, :, :], in_=cs,
                                                 func=AF.Tanh)
                            xi = 2 * br + kp
                            nc.vector.tensor_mul(out=xe16[xi],
                                                 in0=gt[:, 3, :, :],
                                                 in1=gt[:, 2, :, :])
                    # score/q/r matvecs against the 3 packed columns; the
                    # score column goes through sigmoid NOW so the softmax
                    # after the encoder is pure Vector math.
                    for j in range(NJ):
                        psq = psum_q.tile([P, 3], F32, name="psq", tag="psq")
                        for k in range(KD):
                            nc.tensor.matmul(psq,
                                             xe16[k // 2][:, k % 2,
                                                          j * P:(j + 1) * P],
                                             v3t[:, k, :],
                                             start=(k == 0), stop=(k == KD - 1))
                        nc.scalar.activation(out=es[:, j, t:t + 1],
                                             in_=psq[:, 0:1], func=AF.Sigmoid)
                        nc.vector.tensor_copy(out=sq2[:, j, t, :],
                                              in_=psq[:, 1:3])
                    prev16 = xe16

            # decoder weights: emitted here (consumers are decoder-only) but
            # executed by the gpsimd queue during the encoder
            wid3R = pw.tile([P, 8 * H], F16)
            nc.gpsimd.dma_start(out=wid3R, in_=wid3d[:, :])
            whhd = pw.tile([P, KD, 8 * H], F8)
            nc.gpsimd.dma_start(out=whhd, in_=whhdd[:, :, :])

            # ------------- decoder attention (post-encoder, ~2us) ---------
            # keep the PE busy right at the start of the gap
            if upto in ("beta", "dec", "full"):
                _wfill(8)
                _warm_on(ident[0:3 * T, :], ytldT3, 1)
            if upto == "enc":
                nc.vector.tensor_copy(out=osb, in_=es[:, :, 0:1])
                dbgb = pw.tile([P, NJ, T, 3], F32, name="dbgb")
                nc.vector.tensor_copy(out=dbgb[:, :, :, 0], in_=es)
                nc.vector.tensor_copy(out=dbgb[:, :, :, 1:3], in_=sq2)
                nc.sync.dma_start(out=dbgd.rearrange("(j p) t c -> p j t c", p=P),
                                  in_=dbgb)
            for j in range(NJ if upto in ("beta", "dec", "full") else 0):
                # e_t = s/(1-s); beta-weighted sums of q (A) and r (ctxw)
                e = psm.tile([P, T], F32, name="e", tag=f"e{j}")
                om = psm.tile([P, T], F32, name="omb", tag=f"omb{j}")
                nc.vector.tensor_scalar_mul(out=om, in0=es[:, j, :],
                                            scalar1=-1.0)
                nc.vector.tensor_scalar_add(out=om, in0=om, scalar1=1.0)
                nc.vector.reciprocal(out=om, in_=om)
                _warm_on(om, om, 1)
                _wfill(2)
                nc.vector.tensor_mul(out=e, in0=es[:, j, :], in1=om)
                ssum = psm.tile([P, 1], F32, name="ssum", tag=f"bsum{j}")
                nc.vector.reduce_sum(out=ssum, in_=e, axis=mybir.AxisListType.X)
                _warm_on(e, e, 1)
                _wfill(2)
                inv = psm.tile([P, 1], F32, name="inv", tag=f"binv{j}")
                nc.vector.reciprocal(out=inv, in_=ssum)
                tmp9 = psm.tile([P, T], F32, name="tmp9", tag=f"tmp9{j}")
                eq = psm.tile([P, 1], F32, name="eq", tag=f"eq{j}")
                nc.vector.tensor_mul(out=tmp9, in0=e, in1=sq2[:, j, :, 0])
                nc.vector.reduce_sum(out=eq, in_=tmp9, axis=mybir.AxisListType.X)
                nc.vector.tensor_scalar_mul(out=A_t[:, j:j + 1], in0=eq,
                                            scalar1=inv)
                tmp9b = psm.tile([P, T], F32, name="tmp9b", tag=f"tmp9b{j}")
                er = psm.tile([P, 1], F32, name="er", tag=f"er{j}")
                nc.vector.tensor_mul(out=tmp9b, in0=e, in1=sq2[:, j, :, 1])
                nc.vector.reduce_sum(out=er, in_=tmp9b, axis=mybir.AxisListType.X)
                nc.vector.tensor_scalar_mul(out=er, in0=er, scalar1=inv)
                # ctxw = er + fcf_b  (bias folded so the tail is one op)
                nc.vector.tensor_scalar_add(out=ctxw[:, j:j + 1], in0=er,
                                            scalar1=scal["fcf_b"])

            # A row -> the 4 strips of every step tile (36 x 512B DMAs,
            # t-ascending and split across two idle queues, so decoder t=0
            # starts ~1us after A is known)
            if upto in ("dec", "full"):
                nc.vector.tensor_copy(out=A16, in_=A_t)
                arps = psum_q.tile([1, BS], F16, name="arps", tag="psq")
                for j in range(NJ):
                    nc.tensor.transpose(arps[:, j * P:(j + 1) * P],
                                        A16[:, j:j + 1], ident)
                nc.scalar.copy(out=arowT, in_=arps)
                _warm_on(ident[0:1, :], arowT, 1)
                _wfill(2)
                for t in range(T):
                    for g in range(4):
                        eng = nc.sync if (t * 4 + g) % 2 == 0 else nc.gpsimd
                        eng.dma_start(
                            out=ytA[t][32 * g + 1:32 * g + 2, :],
                            in_=arowT)

        if upto == "beta":
            nc.vector.tensor_copy(out=osb, in_=A_t.unsqueeze(2))

        # ---------------- decoder recurrence ----------------
        ndec = T if upto in ("dec", "full") else 0
        dT16p = None
        pdt16 = ctx.enter_context(tc.tile_pool(name="pdt16", bufs=2))
        with tc.tile_pool(name="pgd", bufs=5) as pgd:
            for t in range(ndec):
                if t == 1:
                    _wfill(8)
                dprev16 = dT16p
                dT16p = [pdt16.tile([P, 2, BS], F16, name=f"dT16_{i}",
                                    tag=f"dT16_{i}") for i in range(KD // 2)]
                for kp in range(KD // 2):
                    pss = [psum_g.tile([P, 2, BS], F32, name=f"psd{g}",
                                       tag="psg") for g in range(4)]
                    for half in range(2):
                        for g in range(4):
                            m = g * KD + 2 * kp + half
                            nc.tensor.matmul(
                                pss[g][:, half, :],
                                wid3R[32 * g:32 * g + 3, m * P:(m + 1) * P],
                                ytA[t][32 * g:32 * g + 3, :],
                                start=True, stop=(t == 0),
                                tile_position=(32 * g, 0))
                        if t > 0:
                            for g in range(4):
                                m = g * KD + 2 * kp + half
                                for k in range(KD):
                                    nc.tensor.matmul(
                                        pss[g][:, half, :],
                                        whhd[:, k, m * P:(m + 1) * P],
                                        dprev16[k // 2][:, k % 2, :],
                                        start=False, stop=(k == KD - 1))
                    gt = pgd.tile([P, 4, 2, BS], F16, name="gtd", tag="gd")
                    for g in range(4):
                        fn = AF.Tanh if g == 2 else AF.Sigmoid
                        nc.scalar.activation(out=gt[:, g, :, :], in_=pss[g],
                                             func=fn)
                    cs = cd[:, 2 * kp:2 * kp + 2, :]
                    if t == 0:
                        nc.vector.tensor_mul(out=cs, in0=gt[:, 0, :, :],
                                             in1=gt[:, 2, :, :])
                    else:
                        u = pu.tile([P, 2, BS], F32, name="ud", tag="u")
                        nc.vector.tensor_mul(out=u, in0=gt[:, 0, :, :],
                                             in1=gt[:, 2, :, :])
                        nc.vector.tensor_mul(out=cs, in0=gt[:, 1, :, :], in1=cs)
                        nc.vector.tensor_add(out=cs, in0=cs, in1=u)
                    nc.scalar.activation(out=gt[:, 2, :, :], in_=cs, func=AF.Tanh)
                    nc.vector.tensor_mul(out=dT16p[kp], in0=gt[:, 3, :, :],
                                         in1=gt[:, 2, :, :])

        # ---------------- output ----------------
        if upto == "dec":
            nc.vector.tensor_copy(out=osb, in_=cd[:, 0:NJ, 0:1])
        if upto == "full":
            for j in range(NJ):
                psf = psum_q.tile([P, 1], F32, name="psf", tag="psq")
                for k in range(KD):
                    nc.tensor.matmul(psf,
                                     dT16p[k // 2][:, k % 2, j * P:(j + 1) * P],
                                     wdt[:, k, :],
                                     start=(k == 0), stop=(k == KD - 1))
                nc.vector.scalar_tensor_tensor(
                    out=osb[:, j, :], in0=psf, scalar=1.0,
                    in1=ctxw[:, j:j + 1], op0=ALU.mult, op1=ALU.add)
        nc.sync.dma_start(out=outd.rearrange("(j p) c -> p j c", p=P), in_=osb)

    nc.compile()
    return nc


def _run(inputs, trace=False, upto="full"):
    weights, scal = _pack_weights(inputs)
    nc = _build(scal, upto=upto)
    X = np.ascontiguousarray(_np(inputs["X"]))
    Y = np.ascontiguousarray(_np(inputs["y_prev"]))
    in_maps = []
    for c in range(NCORES):
        m = dict(weights)
        m["x"] = np.ascontiguousarray(X[c * BS:(c + 1) * BS])
        m["y"] = np.ascontiguousarray(Y[c * BS:(c + 1) * BS])
        in_maps.append(m)
    res = run_bass_kernel_spmd(nc, in_maps, core_ids=list(range(NCORES)), trace=trace)
    out = np.concatenate([np.asarray(res.results[i]["out"]) for i in range(NCORES)],
                         axis=0).astype(np.float32)
    return out, res


def kernel(**inputs):
    out, _ = _run(inputs, trace=False)
    return out
